# revision 4
# baseline (speedup 1.0000x reference)
"""EpisodicMemory Trainium2 kernel, v3.

Data-parallel over batch across 8 NeuronCores (128 batch rows per core).

Layout is "flipped": the GRU state h and all gate pre-activations live as
[H-on-partitions (4 k-tiles of 128), batch-on-free(128)] tiles, so the
recurrent matmul h @ W_hh^T needs NO transposes: its rhs (moving operand)
is h itself, and the elementwise update produces h directly in that
layout. Per-sentence episodic gates are broadcast across partitions with a
stride-0 DMA from DRAM.

Precision plan (validated offline, rel err ~9.7e-3 vs 2e-2 budget):
 - x-part (C @ W_ih^T) and scoring fc1: 3-term error-compensated fp8-e4m3
   DoubleRow matmuls (hi/lo splits of both operands, Whi@Chi + Whi@Clo +
   Wlo@Chi), 0.5 cyc/row with K=256 per instruction.
 - h-part (h @ W_hh^T): fp8 DoubleRow with PER-GATE term counts chosen by
   sigmoid/tanh noise sensitivity: r gate 1 term (Whi@hhi), z gate 2 terms
   (Whi@hhi + Whi@hlo), n gate 3 terms. h is kept in fp16 and quantized to
   fp8 hi/lo (scale 1.0, |h|<=1) right on the h' production: the p+m add is
   issued twice (fp8 out first so next-step matmuls unblock early, then
   fp16), plus one subtract for the lo plane.
 - All W matrices pre-scaled so every GRU psum holds 512x the preact
   (fc1: 128x); the 1/512 folds into the ACT sigmoid/tanh scale.
 - C / feat = [C*Q, C*prev_M, |C-Q|, |C-prev_M|] quantized host-side.

Startup: tiny consts DMA first, then fc1 weights + feat group 0 in
interleaved hi/lo quarters so scoring matmuls start ~4us in; identity-tile
warmup matmuls keep the PE busy during the DMA wait so the p-state ramp
(0.65/1.2/2.4 GHz, full speed only after 3us continuous busy) is already
ramped when real work lands.
"""
import numpy as np
import ml_dtypes

H = 512
SH = 120
B = 1024
S = 64
NCORES = 8
BPC = B // NCORES  # 128
KH = H // 128      # 4
G3 = 3 * H
SGRP = 4
NGRP = S // SGRP   # 16
NT = G3 // 128     # 12 gate tiles
FK = 4 * H // 128  # 16 feat k-tiles
E4 = ml_dtypes.float8_e4m3
F16 = np.float16

N_WARMUP = 10      # identity matmuls to ramp the PE p-state during DMA wait

_CACHE = {}


def _q8(x):
    return np.clip(np.asarray(x, np.float32), -240.0, 240.0).astype(E4)


def _split8(x, scale):
    hi = _q8(x * scale)
    lo = _q8(x * scale - hi.astype(np.float32))
    return hi, lo


def _build(consts):
    import concourse.bass as bass
    import concourse.tile as tile
    from concourse import bacc, mybir

    FP32 = mybir.dt.float32
    FP16 = mybir.dt.float16
    FP8 = mybir.dt.float8e4
    OP = mybir.AluOpType
    AF = mybir.ActivationFunctionType
    PM = mybir.MatmulPerfMode

    nc = bacc.Bacc("TRN2", target_bir_lowering=False, debug=False,
                   num_devices=NCORES)

    # ---- external inputs (per core) ----
    c_t = nc.dram_tensor("c8", [2, S, 128, 2, 2, BPC], FP8,
                         kind="ExternalInput")  # [hi/lo, s, p, pair, i, b]
    f_t = nc.dram_tensor("feat8", [2, NGRP, 128, FK // 2, 2, SGRP * BPC],
                         FP8, kind="ExternalInput")
    out = nc.dram_tensor("out", [BPC, H], FP32, kind="ExternalOutput")

    # ---- inline consts ----
    dl = {}
    for k, v in consts.items():
        dl[k] = nc.inline_tensor(v, name=k)

    from contextlib import ExitStack
    with tile.TileContext(nc) as tc:
        with ExitStack() as ctx:
            cpool = ctx.enter_context(tc.tile_pool(name="const", bufs=1))
            cpool2 = ctx.enter_context(tc.tile_pool(name="const2", bufs=1))
            cstr = ctx.enter_context(tc.tile_pool(name="cstr", bufs=6))
            fstr = ctx.enter_context(tc.tile_pool(name="fstr", bufs=3))
            hpool = ctx.enter_context(tc.tile_pool(name="h", bufs=3))
            hq = ctx.enter_context(tc.tile_pool(name="hq", bufs=4))
            gpool = ctx.enter_context(tc.tile_pool(name="g", bufs=4))
            gdram = ctx.enter_context(tc.tile_pool(name="gd", bufs=4,
                                                   space="DRAM"))
            ew = ctx.enter_context(tc.tile_pool(name="ew", bufs=3))
            ew2 = ctx.enter_context(tc.tile_pool(name="ew2", bufs=3))
            sco = ctx.enter_context(tc.tile_pool(name="sco", bufs=2))
            ps_r = ctx.enter_context(tc.tile_pool(name="ps_r", bufs=2,
                                                  space="PSUM"))
            ps_z = ctx.enter_context(tc.tile_pool(name="ps_z", bufs=2,
                                                  space="PSUM"))
            ps_x = ctx.enter_context(tc.tile_pool(name="ps_x", bufs=2,
                                                  space="PSUM"))
            ps_h = ctx.enter_context(tc.tile_pool(name="ps_h", bufs=1,
                                                  space="PSUM"))
            ps_f = ctx.enter_context(tc.tile_pool(name="ps_f", bufs=1,
                                                  space="PSUM"))

            # ---- tiny consts first: identity (warmup + epilogue), fc2 ----
            idt = cpool.tile([128, 128], FP32, tag="idt")
            nc.sync.dma_start(idt[:], dl["ident"].ap())
            f2t = cpool2.tile([SH, 1], FP16, tag="f2t")
            nc.sync.dma_start(f2t[:], dl["f2t16"].ap())
            # touch every activation function once so the ACT table loads
            # happen during the const-DMA wait, not on the scan chain
            warm = cpool2.tile([1, 4], FP32, tag="warm")
            for af in (AF.Sigmoid, AF.Tanh, AF.Copy):
                nc.scalar.activation(warm[:], idt[0:1, 0:4], af)
            # PE p-state warmup: fp32 identity matmuls (512 cyc each), no
            # data deps beyond idt, so they run while the big DMAs stream.
            # Aliases the fc1 psum tag: PE is in-order, so no extra blocking.
            wps = ps_f.tile([128, SGRP * BPC], FP32, tag="pps", name="wps")
            for i in range(N_WARMUP):
                nc.tensor.matmul(wps[:, 0:128], idt[:], idt[:],
                                 start=(i == 0), stop=(i == N_WARMUP - 1))

            # ---- fc1 weights, then feat g0 (quartered) so scoring starts
            # as early as possible ----
            f1h = cpool2.tile([128, FK // 2, 2, 128], FP8, tag="f1h")
            nc.sync.dma_start(f1h[:], dl["f18h"].ap())
            f1l = cpool2.tile([128, FK // 2, 2, 128], FP8, tag="f1l")
            nc.sync.dma_start(f1l[:], dl["f18l"].ap())

            # ================= helpers =================
            def load_c(s):
                ch = cstr.tile([128, 2, 2, BPC], FP8, tag="csh")
                nc.sync.dma_start(ch[:], c_t.ap()[0, s])
                cl = cstr.tile([128, 2, 2, BPC], FP8, tag="csl")
                nc.sync.dma_start(cl[:], c_t.ap()[1, s])
                return (ch, cl)

            def x_mms(cts, banks, with_stop):
                """x-part matmuls for one sentence into the given
                [(psum, gate)] banks. start on each bank's first matmul,
                stop on its last iff with_stop or the bank is pxn (gate 2,
                which the h-part never accumulates into)."""
                ch, cl = cts
                for pb, gate in banks:
                    for jj in range(4):
                        gt_ = gate * 4 + jj
                        for pair in range(2):
                            terms = [(wih_h, ch), (wih_h, cl),
                                     (wih_l, ch)]
                            for ti, (wt_, ct_) in enumerate(terms):
                                nc.tensor.matmul(
                                    pb[:, jj],
                                    wt_[:, pair, gt_],
                                    ct_[:, pair],
                                    start=(jj == 0 and pair == 0
                                           and ti == 0),
                                    stop=((with_stop or gate == 2)
                                          and jj == 3
                                          and pair == 1 and ti == 2),
                                    perf_mode=PM.DoubleRow)

            def h_mms(h8h_t, h8l_t, pr, pz, pxn, phn):
                """h-part fp8 DR matmuls, per-gate term counts (r1/z2/n3).
                Ordered so instrs needing only the hi plane come first and
                pair 1 (kt 2,3 — produced first by the chain) leads."""
                # r: 1 term (Whi @ hhi)
                for pair in (1, 0):
                    for jj in range(4):
                        nc.tensor.matmul(
                            pr[:, jj], whh_h[:, pair, jj],
                            h8h_t[:, 2 * pair:2 * pair + 2],
                            start=False, stop=(pair == 0 and jj == 3),
                            perf_mode=PM.DoubleRow)
                # n: 3 terms; the hlo-consuming term last
                for ti, (wt_, ht_) in enumerate(
                        [(whh_h, h8h_t), (whh_l, h8h_t), (whh_h, h8l_t)]):
                    for pair in (1, 0):
                        for jj in range(4):
                            nc.tensor.matmul(
                                phn[:, jj], wt_[:, pair, 8 + jj],
                                ht_[:, 2 * pair:2 * pair + 2],
                                start=(ti == 0 and pair == 1 and jj == 0),
                                stop=(ti == 2 and pair == 0 and jj == 3),
                                perf_mode=PM.DoubleRow)
                # z: 2 terms (Whi @ hhi + Whi @ hlo)
                for ti, ht_ in enumerate([h8h_t, h8l_t]):
                    for pair in (1, 0):
                        for jj in range(4):
                            nc.tensor.matmul(
                                pz[:, jj], whh_h[:, pair, 4 + jj],
                                ht_[:, 2 * pair:2 * pair + 2],
                                start=False,
                                stop=(ti == 1 and pair == 0 and jj == 3),
                                perf_mode=PM.DoubleRow)

            # ---- scoring machinery ----
            grp = {}

            def load_feat(gi, quarters=False):
                # split loads along k so downstream fc1 chunks unblock
                # progressively; interleave hi/lo quarters for the startup
                # groups so chunk q unblocks after 1/4 of the bytes
                fh = fstr.tile([128, FK // 2, 2, SGRP * BPC], FP8,
                               tag="feath")
                fl = fstr.tile([128, FK // 2, 2, SGRP * BPC], FP8,
                               tag="featl")
                if quarters:
                    for q0 in range(0, 8, 2):
                        nc.sync.dma_start(fh[:, q0:q0 + 2],
                                          f_t.ap()[0, gi, :, q0:q0 + 2])
                        nc.sync.dma_start(fl[:, q0:q0 + 2],
                                          f_t.ap()[1, gi, :, q0:q0 + 2])
                else:
                    nc.sync.dma_start(fh[:, 0:4], f_t.ap()[0, gi, :, 0:4])
                    nc.sync.dma_start(fh[:, 4:8], f_t.ap()[0, gi, :, 4:8])
                    nc.sync.dma_start(fl[:, 0:4], f_t.ap()[1, gi, :, 0:4])
                    nc.sync.dma_start(fl[:, 4:8], f_t.ap()[1, gi, :, 4:8])
                grp[gi] = {"feat": (fh, fl)}

            def fc1_chunk(gi, q):
                """Quarter q of group gi's fc1 matmuls."""
                st = grp[gi]
                if q == 0:
                    st["pps"] = ps_f.tile([128, SGRP * BPC], FP32, tag="pps",
                                          name="pps")
                pps = st["pps"]
                fh, fl = st["feat"]
                for pair in range(2 * q, 2 * q + 2):
                    terms = [(f1h, fh), (f1h, fl), (f1l, fh)]
                    for ti, (wt_, ft_) in enumerate(terms):
                        nc.tensor.matmul(
                            pps[:], wt_[:, pair], ft_[:, pair],
                            start=(pair == 0 and ti == 0),
                            stop=(pair == FK // 2 - 1 and ti == 2),
                            perf_mode=PM.DoubleRow)
                if q == 3:
                    finish_group(gi)

            def finish_group(gi):
                st = grp[gi]
                pps = st["pps"]
                h1 = sco.tile([SH, SGRP * BPC], FP16, tag="h1")
                nc.scalar.activation(h1[:], pps[0:SH, :], AF.Tanh,
                                     scale=1.0 / 128)
                nc.tensor.matmul(pps[0:1, :], f2t[:], h1[:],
                                 start=True, stop=True)
                gt = gpool.tile([1, SGRP * BPC], FP16, tag="gt")
                nc.scalar.activation(gt[:], pps[0:1, :], AF.Sigmoid)
                gd = gdram.tile([1, SGRP * BPC], FP16, tag="gd")
                nc.sync.dma_start(gd[:], gt[:])
                grep = gpool.tile([128, SGRP, BPC], FP16, tag="grep")
                nc.sync.dma_start(
                    grep[:], gd[:].broadcast_to([128, SGRP * BPC]))
                st["grep"] = grep
                del st["pps"], st["feat"]

            # ================= prologue =================
            load_feat(0, quarters=True)
            for q in range(4):
                fc1_chunk(0, q)

            # x weights + first sentence, then x matmuls for s=0
            # (no r needed: h=0 -> r*hn=0)
            wih_h = cpool.tile([128, 2, NT, 2, 128], FP8, tag="wih_h")
            nc.sync.dma_start(wih_h[:], dl["wih8h"].ap())
            wih_l = cpool.tile([128, 2, NT, 2, 128], FP8, tag="wih_l")
            nc.sync.dma_start(wih_l[:], dl["wih8l"].ap())
            cbuf = {0: load_c(0)}
            pz = ps_z.tile([128, 4, BPC], FP32, tag="pz", name="pz")
            pxn = ps_x.tile([128, 4, BPC], FP32, tag="pxn", name="pxn")
            x_mms(cbuf[0], [(pxn, 2), (pz, 1)], with_stop=True)
            del cbuf[0]

            load_feat(1, quarters=True)
            for q in range(4):
                fc1_chunk(1, q)

            # recurrent weights (fp8 hi/lo, DR lhsT layout like wih)
            whh_h = cpool.tile([128, 2, NT, 2, 128], FP8, tag="whh_h")
            nc.sync.dma_start(whh_h[:], dl["whh8h"].ap())
            whh_l = cpool.tile([128, 2, NT, 2, 128], FP8, tag="whh_l")
            nc.sync.dma_start(whh_l[:], dl["whh8l"].ap())

            for s in range(1, 5):
                cbuf[s] = load_c(s)
            load_feat(2)

            h16 = None
            h8h_t = None
            h8l_t = None
            pr = None
            ISC = 1.0 / 512

            # ================= scan =================
            for s in range(S):
                # ---- h-part matmuls (s>0) ----
                if s > 0:
                    phn = ps_h.tile([128, 4, BPC], FP32, tag="phn",
                                    name="phn")
                    h_mms(h8h_t, h8l_t, pr, pz, pxn, phn)

                # ---- elementwise chain ----
                gi = s // SGRP
                j = s % SGRP
                grep = grp[gi]["grep"]

                if s > 0:
                    r_sb = ew.tile([128, 4, BPC], FP16, tag="r")
                    nc.scalar.activation(r_sb[:], pr[:], AF.Sigmoid,
                                         scale=ISC)
                    tn = ew2.tile([128, 4, BPC], FP32, tag="tn")
                    nc.vector.tensor_tensor(tn[:], r_sb[:], phn[:], OP.mult)
                w_sb = ew.tile([128, 4, BPC], FP16, tag="w")
                nc.scalar.activation(w_sb[:], pz[:], AF.Sigmoid, scale=-ISC)
                if s > 0:
                    tn2 = ew2.tile([128, 4, BPC], FP32, tag="tn2")
                    nc.vector.tensor_tensor(tn2[:], tn[:], pxn[:], OP.add)
                a_sb = ew.tile([128, 4, BPC], FP16, tag="a")
                nc.vector.tensor_tensor(
                    a_sb[:], w_sb[:],
                    grep[:, j].unsqueeze(1).broadcast_to([128, 4, BPC]),
                    OP.mult)
                # p = (1-a)*h, computed OFF the critical chain (a and h are
                # both ready before tanh) so the post-tanh tail is only
                # m2 = a*n ; h' = p + m2
                if s > 0:
                    t1 = ew2.tile([128, 4, BPC], FP16, tag="t1")
                    nc.vector.tensor_tensor(t1[:], a_sb[:], h16[:], OP.mult)
                    p_sb = ew2.tile([128, 4, BPC], FP16, tag="p")
                    nc.vector.tensor_tensor(p_sb[:], h16[:], t1[:],
                                            OP.subtract)
                n_sb = ew.tile([128, 4, BPC], FP16, tag="n")
                if s > 0:
                    nc.scalar.activation(n_sb[:], tn2[:], AF.Tanh, scale=ISC)
                else:
                    nc.scalar.activation(n_sb[:], pxn[:], AF.Tanh, scale=ISC)

                last = s == S - 1
                if last:
                    nh = ew.tile([128, KH, BPC], FP32, tag="hf")
                    nhh = nhl = None
                else:
                    nh = hpool.tile([128, KH, BPC], FP16, tag="h", name="h")
                    nhh = hq.tile([128, KH, BPC], FP8, tag="h8h",
                                  name="h8h")
                    nhl = hq.tile([128, KH, BPC], FP8, tag="h8l",
                                  name="h8l")
                for half in (1, 0):
                    k0 = 2 * half
                    if s == 0:
                        if not last:
                            nc.vector.tensor_tensor(
                                nhh[:, k0:k0 + 2], a_sb[:, k0:k0 + 2],
                                n_sb[:, k0:k0 + 2], OP.mult)
                        nc.vector.tensor_tensor(
                            nh[:, k0:k0 + 2], a_sb[:, k0:k0 + 2],
                            n_sb[:, k0:k0 + 2], OP.mult)
                    else:
                        m_h = ew2.tile([128, 2, BPC], FP16, tag=f"m{half}",
                                       name=f"m{half}")
                        nc.vector.tensor_tensor(
                            m_h[:], a_sb[:, k0:k0 + 2], n_sb[:, k0:k0 + 2],
                            OP.mult)
                        # fp8 hi plane first: next step's r matmuls need
                        # only this
                        if not last:
                            nc.vector.tensor_tensor(
                                nhh[:, k0:k0 + 2], p_sb[:, k0:k0 + 2],
                                m_h[:], OP.add)
                        nc.vector.tensor_tensor(
                            nh[:, k0:k0 + 2], p_sb[:, k0:k0 + 2], m_h[:],
                            OP.add)
                    if not last:
                        nc.vector.tensor_tensor(
                            nhl[:, k0:k0 + 2], nh[:, k0:k0 + 2],
                            nhh[:, k0:k0 + 2], OP.subtract)
                h16 = nh
                h8h_t = nhh
                h8l_t = nhl

                # ---- x-part matmuls for s+1 ----
                if s + 1 < S:
                    pr = ps_r.tile([128, 4, BPC], FP32, tag="pr", name="pr")
                    pz = ps_z.tile([128, 4, BPC], FP32, tag="pz", name="pz")
                    pxn = ps_x.tile([128, 4, BPC], FP32, tag="pxn",
                                    name="pxn")
                    x_mms(cbuf[s + 1], [(pr, 0), (pxn, 2), (pz, 1)],
                          with_stop=False)
                    del cbuf[s + 1]
                    if s + 5 < S:
                        cbuf[s + 5] = load_c(s + 5)

                # ---- interleaved scoring ----
                gisc = s // SGRP + 2
                q = s % SGRP
                if gisc <= NGRP - 1:
                    if q == 0 and gisc + 1 <= NGRP - 1:
                        load_feat(gisc + 1)
                    fc1_chunk(gisc, q)

            # ================= epilogue =================
            pt = ps_r.tile([128, 4, BPC], FP32, tag="pr", name="ptr")
            for kt in range(KH):
                nc.tensor.transpose(pt[:, kt], h16[:, kt], idt[:])
            ot = ew.tile([128, H], FP32, tag="ot")
            nc.scalar.activation(ot[:], pt[:].rearrange("p k b -> p (k b)"),
                                 AF.Copy)
            nc.sync.dma_start(out.ap(), ot[:])

    nc.compile()
    return nc


def _prep(C, Q, prev_M, fc1_w, fc2_w, W_ih, W_hh):
    """Host-side layout/dtype transforms + per-core sharding."""
    consts = {}
    consts["ident"] = np.eye(128, dtype=np.float32)
    # W_ih^T hi/lo fp8 in DR lhsT layout [p, pair, gate-tile, i, 128]
    hi, lo = _split8(np.ascontiguousarray(W_ih.T), 16.0)  # [K, G3]
    for nm, t in (("wih8h", hi), ("wih8l", lo)):
        consts[nm] = np.ascontiguousarray(
            t.reshape(2, 2, 128, NT, 128).transpose(2, 0, 3, 1, 4))
    # W_hh^T * 512 hi/lo fp8, same layout (h quantized at scale 1.0)
    hi, lo = _split8(np.ascontiguousarray(W_hh.T), 512.0)  # [H, G3]
    for nm, t in (("whh8h", hi), ("whh8l", lo)):
        consts[nm] = np.ascontiguousarray(
            t.reshape(2, 2, 128, NT, 128).transpose(2, 0, 3, 1, 4))
    f1p = np.zeros((128, 4 * H), np.float32)  # pad SH 120 -> 128
    f1p[:SH] = fc1_w
    hi, lo = _split8(np.ascontiguousarray(f1p.T), 16.0)  # [K, 128]
    for nm, t in (("f18h", hi), ("f18l", lo)):
        consts[nm] = np.ascontiguousarray(
            t.reshape(FK // 2, 2, 128, 128).transpose(2, 0, 1, 3))
    consts["f2t16"] = np.ascontiguousarray(fc2_w.T).astype(F16)

    in_maps = []
    for c in range(NCORES):
        lo_, hi_ = c * BPC, (c + 1) * BPC
        Cc = np.ascontiguousarray(C[lo_:hi_])          # [BPC, S, H]
        Qc = Q[lo_:hi_, 0]                              # [BPC, H]
        Mc = prev_M[lo_:hi_, 0]
        m = {}
        Ct = np.ascontiguousarray(Cc.transpose(1, 2, 0))   # [S, H, BPC]
        chi, clo = _split8(Ct, 32.0)  # [S, H, BPC]
        both = np.stack([chi, clo])   # [2, S, H, BPC]
        m["c8"] = np.ascontiguousarray(
            both.reshape(2, S, 2, 2, 128, BPC).transpose(0, 1, 4, 2, 3, 5))
        # feat [BPC, S, 4H]
        feat = np.concatenate(
            [Cc * Qc[:, None, :], Cc * Mc[:, None, :],
             np.abs(Cc - Qc[:, None, :]), np.abs(Cc - Mc[:, None, :])],
            axis=2)
        # -> [NGRP, 4H, SGRP, BPC]
        ftr = np.ascontiguousarray(
            feat.transpose(1, 2, 0).reshape(NGRP, SGRP, 4 * H, BPC)
            .transpose(0, 2, 1, 3))
        fhi, flo = _split8(ftr, 8.0)       # [NGRP, 4H, SGRP, BPC]
        both = np.stack([fhi, flo])
        m["feat8"] = np.ascontiguousarray(
            both.reshape(2, NGRP, FK // 2, 2, 128, SGRP * BPC)
            .transpose(0, 1, 4, 2, 3, 5))
        in_maps.append(m)
    return consts, in_maps


def kernel(C, Q, prev_M, fc1_w, fc1_b, fc2_w, fc2_b, W_ih, W_hh, b_ih, b_hh):
    from concourse.bass_utils import run_bass_kernel_spmd

    C = np.asarray(C, dtype=np.float32)
    Q = np.asarray(Q, dtype=np.float32)
    prev_M = np.asarray(prev_M, dtype=np.float32)
    fc1_w = np.asarray(fc1_w, np.float32)
    fc2_w = np.asarray(fc2_w, np.float32)
    W_ih = np.asarray(W_ih, np.float32)
    W_hh = np.asarray(W_hh, np.float32)
    fc1_b = np.asarray(fc1_b, np.float32)
    fc2_b = np.asarray(fc2_b, np.float32)
    b_ih = np.asarray(b_ih, np.float32)
    b_hh = np.asarray(b_hh, np.float32)
    assert not (np.any(fc1_b) or np.any(fc2_b) or np.any(b_ih)
                or np.any(b_hh)), "nonzero biases unsupported in v3"

    consts, in_maps = _prep(C, Q, prev_M, fc1_w, fc2_w, W_ih, W_hh)

    key = tuple(np.asarray(v).tobytes() for v in consts.values())
    kh = hash(key)
    if kh not in _CACHE:
        _CACHE[kh] = _build(consts)
    nc = _CACHE[kh]

    res = run_bass_kernel_spmd(nc, in_maps, list(range(NCORES)))
    h = np.concatenate([res.results[c]["out"] for c in range(NCORES)],
                       axis=0)
    return h[:, None, :].astype(np.float32)


# revision 37
# speedup vs baseline: 1.1545x; 1.1545x over previous
"""EpisodicMemory Trainium2 kernel, v3.

Data-parallel over batch across 8 NeuronCores (128 batch rows per core).

Layout is "flipped": the GRU state h and all gate pre-activations live as
[H-on-partitions (4 k-tiles of 128), batch-on-free(128)] tiles, so the
recurrent matmul h @ W_hh^T needs NO transposes: its rhs (moving operand)
is h itself, and the elementwise update produces h directly in that
layout. Per-sentence episodic gates are broadcast across partitions with a
stride-0 DMA from DRAM.

Precision plan (validated offline, rel err ~1.3e-2 vs 2e-2 budget):
 - x-part (C @ W_ih^T) and scoring fc1: 3-term error-compensated fp8-e4m3
   DoubleRow matmuls (hi/lo splits of both operands, Whi@Chi + Whi@Clo +
   Wlo@Chi), 0.5 cyc/row with K=256 per instruction.
 - h-part (h @ W_hh^T): fp8 DoubleRow, every gate 2-term WEIGHT-side
   compensated ((Whi+Wlo) @ hhi) so only the hi fp8 plane of h is ever
   needed. h stays in fp16 (quantized at scale 1.0, |h|<=1); the chain's
   p+m add is issued twice, fp8-out first (h8h) so the next step's h-part
   matmuls unblock before the fp16 adds complete.
 - tn/tn2 (r*hn + xn) are fp16 at the 512x psum scale; the n-gate x psum
   is pre-staged to fp16 SBUF (ACT copy, off the critical path) so tn2
   runs in the DVE 2x mode instead of reading psum at 1x.
 - All W matrices pre-scaled so every GRU psum holds 512x the preact
   (fc1: 128x); the 1/512 folds into the ACT sigmoid/tanh scale.
 - C / feat = [C*Q, C*prev_M, |C-Q|, |C-prev_M|] quantized host-side.

Per-step critical loop (the scan is latency-bound, not engine-bound):
h8h -> r/n/z h-matmuls -> sigmoid(r) -> tn -> tn2 -> tanh (2 chunks) ->
m -> h8h. DVE work is kept just under the loop period; a/t1/p fill the
tanh window. Startup: tiny consts DMA first, then fc1 weights + feat
group 0 in interleaved hi/lo quarters so scoring matmuls start ~4us in;
identity-tile warmup matmuls ramp the PE p-state during the DMA wait.
"""
import numpy as np
import ml_dtypes

H = 512
SH = 120
B = 1024
S = 64
NCORES = 8
BPC = B // NCORES  # 128
KH = H // 128      # 4
G3 = 3 * H
SGRP = 4
NGRP = S // SGRP   # 16
NT = G3 // 128     # 12 gate tiles
FK = 4 * H // 128  # 16 feat k-tiles
E4 = ml_dtypes.float8_e4m3
F16 = np.float16

N_WARMUP = 10      # identity matmuls to ramp the PE p-state during DMA wait

_CACHE = {}


def _q8(x):
    return np.clip(np.asarray(x, np.float32), -240.0, 240.0).astype(E4)


def _split8(x, scale):
    hi = _q8(x * scale)
    lo = _q8(x * scale - hi.astype(np.float32))
    return hi, lo


def _build(consts):
    import concourse.bass as bass
    import concourse.tile as tile
    from concourse import bacc, mybir

    FP32 = mybir.dt.float32
    FP16 = mybir.dt.float16
    FP8 = mybir.dt.float8e4
    OP = mybir.AluOpType
    AF = mybir.ActivationFunctionType
    PM = mybir.MatmulPerfMode

    nc = bacc.Bacc("TRN2", target_bir_lowering=False, debug=False,
                   num_devices=NCORES)

    # ---- external inputs (per core) ----
    c_t = nc.dram_tensor("c8", [2, S, 128, 2, 2, BPC], FP8,
                         kind="ExternalInput")  # [hi/lo, s, p, pair, i, b]
    f_t = nc.dram_tensor("feat8", [2, NGRP, 128, FK // 2, 2, SGRP * BPC],
                         FP8, kind="ExternalInput")
    out = nc.dram_tensor("out", [BPC, H], FP32, kind="ExternalOutput")

    # ---- inline consts ----
    dl = {}
    for k, v in consts.items():
        dl[k] = nc.inline_tensor(v, name=k)

    from contextlib import ExitStack
    with tile.TileContext(nc) as tc:
        with ExitStack() as ctx:
            cpool = ctx.enter_context(tc.tile_pool(name="const", bufs=1))
            cpool2 = ctx.enter_context(tc.tile_pool(name="const2", bufs=1))
            cstr = ctx.enter_context(tc.tile_pool(name="cstr", bufs=6))
            fstr = ctx.enter_context(tc.tile_pool(name="fstr", bufs=3))
            hpool = ctx.enter_context(tc.tile_pool(name="h", bufs=3))
            hq = ctx.enter_context(tc.tile_pool(name="hq", bufs=4))
            gpool = ctx.enter_context(tc.tile_pool(name="g", bufs=4))
            gdram = ctx.enter_context(tc.tile_pool(name="gd", bufs=4,
                                                   space="DRAM"))
            ew = ctx.enter_context(tc.tile_pool(name="ew", bufs=3))
            ew2 = ctx.enter_context(tc.tile_pool(name="ew2", bufs=3))
            xnp = ctx.enter_context(tc.tile_pool(name="xn16", bufs=3))
            sco = ctx.enter_context(tc.tile_pool(name="sco", bufs=2))
            ps_r = ctx.enter_context(tc.tile_pool(name="ps_r", bufs=2,
                                                  space="PSUM"))
            ps_z = ctx.enter_context(tc.tile_pool(name="ps_z", bufs=2,
                                                  space="PSUM"))
            ps_x = ctx.enter_context(tc.tile_pool(name="ps_x", bufs=2,
                                                  space="PSUM"))
            ps_h = ctx.enter_context(tc.tile_pool(name="ps_h", bufs=1,
                                                  space="PSUM"))
            ps_f = ctx.enter_context(tc.tile_pool(name="ps_f", bufs=1,
                                                  space="PSUM"))

            # ---- tiny consts first: identity (warmup + epilogue), fc2 ----
            idt = cpool.tile([128, 128], FP32, tag="idt")
            nc.sync.dma_start(idt[:], dl["ident"].ap())
            f2t = cpool2.tile([SH, 1], FP16, tag="f2t")
            nc.sync.dma_start(f2t[:], dl["f2t16"].ap())
            # touch every activation function once so the ACT table loads
            # happen during the const-DMA wait, not on the scan chain
            warm = cpool2.tile([1, 4], FP32, tag="warm")
            for af in (AF.Sigmoid, AF.Tanh, AF.Copy):
                nc.scalar.activation(warm[:], idt[0:1, 0:4], af)
            # PE p-state warmup: fp32 identity matmuls (512 cyc each), no
            # data deps beyond idt, so they run while the big DMAs stream.
            # Aliases the fc1 psum tag: PE is in-order, so no extra blocking.
            wps = ps_f.tile([128, SGRP * BPC], FP32, tag="pps", name="wps")
            for i in range(N_WARMUP):
                nc.tensor.matmul(wps[:, 0:128], idt[:], idt[:],
                                 start=(i == 0), stop=(i == N_WARMUP - 1))

            # ---- fc1 weights, then feat g0 (quartered) so scoring starts
            # as early as possible ----
            f1h = cpool2.tile([128, FK // 2, 2, 128], FP8, tag="f1h")
            nc.sync.dma_start(f1h[:], dl["f18h"].ap())
            f1l = cpool2.tile([128, FK // 2, 2, 128], FP8, tag="f1l")
            nc.sync.dma_start(f1l[:], dl["f18l"].ap())

            # ================= helpers =================
            def load_c(s):
                ch = cstr.tile([128, 2, 2, BPC], FP8, tag="csh")
                nc.sync.dma_start(ch[:], c_t.ap()[0, s])
                cl = cstr.tile([128, 2, 2, BPC], FP8, tag="csl")
                nc.sync.dma_start(cl[:], c_t.ap()[1, s])
                return (ch, cl)

            def x_mms(cts, banks, with_stop):
                """x-part matmuls for one sentence into the given
                [(psum, gate)] banks. pr/pz use per-jj-half accumulation
                regions (the chain's sigmoid chunks read each half as soon
                as the h-part closes it); pxn is a single region, closed
                here (the h-part never accumulates into it)."""
                ch, cl = cts
                for pb, gate in banks:
                    for jj in range(4):
                        gt_ = gate * 4 + jj
                        for pair in range(2):
                            terms = [(wih_h, ch), (wih_h, cl),
                                     (wih_l, ch)]
                            for ti, (wt_, ct_) in enumerate(terms):
                                nc.tensor.matmul(
                                    pb[:, jj],
                                    wt_[:, pair, gt_],
                                    ct_[:, pair],
                                    start=(jj == 0 and pair == 0
                                           and ti == 0),
                                    stop=((with_stop or gate == 2)
                                          and jj == 3
                                          and pair == 1 and ti == 2),
                                    perf_mode=PM.DoubleRow)

            def h_mms(h8h_t, pr, pz, pxn, phn):
                """h-part fp8 DR matmuls: every gate is 2-term weight-side
                compensated ((Whi+Wlo) @ hhi), so only the hi plane of h is
                needed — the chain never produces a lo plane and the n-gate
                psum closes right after the hi plane lands. Gate order
                r, n, z: r feeds the sigmoid, n closes phn for tn, z last."""
                for gate, pb in ((0, pr), (2, phn), (1, pz)):
                    for ti, wt_ in enumerate([whh_h, whh_l]):
                        for pair in (1, 0):
                            for jj in range(4):
                                nc.tensor.matmul(
                                    pb[:, jj], wt_[:, pair, 4 * gate + jj],
                                    h8h_t[:, 2 * pair:2 * pair + 2],
                                    start=(gate == 2 and ti == 0
                                           and pair == 1 and jj == 0),
                                    stop=(ti == 1 and pair == 0
                                          and jj == 3),
                                    perf_mode=PM.DoubleRow)

            # ---- scoring machinery ----
            grp = {}

            def load_feat(gi, quarters=False):
                # split loads along k so downstream fc1 chunks unblock
                # progressively; interleave hi/lo quarters for the startup
                # groups so chunk q unblocks after 1/4 of the bytes
                fh = fstr.tile([128, FK // 2, 2, SGRP * BPC], FP8,
                               tag="feath")
                fl = fstr.tile([128, FK // 2, 2, SGRP * BPC], FP8,
                               tag="featl")
                if quarters:
                    for q0 in range(0, 8, 2):
                        nc.sync.dma_start(fh[:, q0:q0 + 2],
                                          f_t.ap()[0, gi, :, q0:q0 + 2])
                        nc.sync.dma_start(fl[:, q0:q0 + 2],
                                          f_t.ap()[1, gi, :, q0:q0 + 2])
                else:
                    nc.sync.dma_start(fh[:, 0:4], f_t.ap()[0, gi, :, 0:4])
                    nc.sync.dma_start(fh[:, 4:8], f_t.ap()[0, gi, :, 4:8])
                    nc.sync.dma_start(fl[:, 0:4], f_t.ap()[1, gi, :, 0:4])
                    nc.sync.dma_start(fl[:, 4:8], f_t.ap()[1, gi, :, 4:8])
                grp[gi] = {"feat": (fh, fl)}

            def fc1_chunk(gi, q):
                """Quarter q of group gi's fc1 matmuls."""
                st = grp[gi]
                if q == 0:
                    st["pps"] = ps_f.tile([128, SGRP * BPC], FP32, tag="pps",
                                          name="pps")
                pps = st["pps"]
                fh, fl = st["feat"]
                for pair in range(2 * q, 2 * q + 2):
                    terms = [(f1h, fh), (f1h, fl), (f1l, fh)]
                    for ti, (wt_, ft_) in enumerate(terms):
                        nc.tensor.matmul(
                            pps[:], wt_[:, pair], ft_[:, pair],
                            start=(pair == 0 and ti == 0),
                            stop=(pair == FK // 2 - 1 and ti == 2),
                            perf_mode=PM.DoubleRow)
                if q == 3:
                    finish_group(gi)

            def finish_group(gi):
                st = grp[gi]
                pps = st["pps"]
                h1 = sco.tile([SH, SGRP * BPC], FP16, tag="h1")
                nc.scalar.activation(h1[:], pps[0:SH, :], AF.Tanh,
                                     scale=1.0 / 128)
                nc.tensor.matmul(pps[0:1, :], f2t[:], h1[:],
                                 start=True, stop=True)
                gt = gpool.tile([1, SGRP * BPC], FP16, tag="gt")
                nc.scalar.activation(gt[:], pps[0:1, :], AF.Sigmoid)
                gd = gdram.tile([1, SGRP * BPC], FP16, tag="gd")
                nc.sync.dma_start(gd[:], gt[:])
                grep = gpool.tile([128, SGRP, BPC], FP16, tag="grep")
                nc.sync.dma_start(
                    grep[:], gd[:].broadcast_to([128, SGRP * BPC]))
                st["grep"] = grep
                del st["pps"], st["feat"]

            # ================= prologue =================
            load_feat(0, quarters=True)
            for q in range(4):
                fc1_chunk(0, q)

            wih_h = cpool.tile([128, 2, NT, 2, 128], FP8, tag="wih_h")
            nc.sync.dma_start(wih_h[:], dl["wih8h"].ap())
            wih_l = cpool.tile([128, 2, NT, 2, 128], FP8, tag="wih_l")
            nc.sync.dma_start(wih_l[:], dl["wih8l"].ap())
            cbuf = {0: load_c(0)}
            pz = ps_z.tile([128, 4, BPC], FP32, tag="pz", name="pz")
            pxn = ps_x.tile([128, 4, BPC], FP32, tag="pxn", name="pxn")
            x_mms(cbuf[0], [(pxn, 2), (pz, 1)], with_stop=True)
            del cbuf[0]

            load_feat(1, quarters=True)
            for q in range(4):
                fc1_chunk(1, q)

            # recurrent weights (fp8 hi/lo, DR lhsT layout like wih);
            # hi plane first — it alone unblocks step 1's r matmuls
            whh_h = cpool.tile([128, 2, NT, 2, 128], FP8, tag="whh_h")
            nc.sync.dma_start(whh_h[:], dl["whh8h"].ap())
            whh_l = cpool.tile([128, 2, NT, 2, 128], FP8, tag="whh_l")
            nc.sync.dma_start(whh_l[:], dl["whh8l"].ap())

            for s in range(1, 5):
                cbuf[s] = load_c(s)
            load_feat(2)

            h16 = None
            h8h_t = None
            pr = None
            ISC = 1.0 / 512

            # ================= scan =================
            for s in range(S):
                # ---- h-part matmuls (s>0) ----
                if s > 0:
                    phn = ps_h.tile([128, 4, BPC], FP32, tag="phn",
                                    name="phn")
                    h_mms(h8h_t, pr, pz, pxn, phn)

                # ---- elementwise chain ----
                gi = s // SGRP
                j = s % SGRP
                grep = grp[gi]["grep"]

                if s > 0:
                    r_sb = ew.tile([128, 4, BPC], FP16, tag="r")
                    nc.scalar.activation(r_sb[:], pr[:], AF.Sigmoid,
                                         scale=ISC)
                    # tn/tn2 in fp16 at the 512x psum scale: tn2's second
                    # operand is the pre-copied fp16 xn, so the add runs in
                    # DVE fast mode instead of reading psum at 1x
                    tn = ew2.tile([128, 4, BPC], FP16, tag="tn")
                    nc.vector.tensor_tensor(tn[:], r_sb[:], phn[:], OP.mult)
                w_sb = ew.tile([128, 4, BPC], FP16, tag="w")
                nc.scalar.activation(w_sb[:], pz[:], AF.Sigmoid, scale=-ISC)
                if s > 0:
                    tn2 = ew2.tile([128, 4, BPC], FP16, tag="tn2")
                    nc.vector.tensor_tensor(tn2[:], tn[:], x16n[:], OP.add)
                a_sb = ew.tile([128, 4, BPC], FP16, tag="a")
                nc.vector.tensor_tensor(
                    a_sb[:], w_sb[:],
                    grep[:, j].unsqueeze(1).broadcast_to([128, 4, BPC]),
                    OP.mult)
                # a / t1 / p are off the critical chain (they fill the DVE
                # while tanh runs on ACT)
                if s > 0:
                    t1 = ew2.tile([128, 4, BPC], FP16, tag="t1")
                    nc.vector.tensor_tensor(t1[:], a_sb[:], h16[:], OP.mult)
                    p_sb = ew2.tile([128, 4, BPC], FP16, tag="p")
                    nc.vector.tensor_tensor(p_sb[:], h16[:], t1[:],
                                            OP.subtract)
                n_sb = ew.tile([128, 4, BPC], FP16, tag="n")
                CHT = ((2, 4), (0, 2))
                for c0_, c1_ in CHT:
                    if s > 0:
                        nc.scalar.activation(n_sb[:, c0_:c1_],
                                             tn2[:, c0_:c1_], AF.Tanh,
                                             scale=ISC)
                    else:
                        nc.scalar.activation(n_sb[:, c0_:c1_],
                                             pxn[:, c0_:c1_], AF.Tanh,
                                             scale=ISC)
                last = s == S - 1
                if last:
                    nh = ew.tile([128, KH, BPC], FP32, tag="hf")
                    nhh = None
                else:
                    nh = hpool.tile([128, KH, BPC], FP16, tag="h", name="h")
                    nhh = hq.tile([128, KH, BPC], FP8, tag="h8h",
                                  name="h8h")
                # post-tanh tail, all on DVE (no cross-engine hops): the fp8
                # hi-plane adds come FIRST — they alone unblock ALL of next
                # step's h-part matmuls; the fp16 adds trail
                mh = {}
                for half in (1, 0):
                    k0 = 2 * half
                    if s > 0:
                        m_h = ew2.tile([128, 2, BPC], FP16, tag=f"m{half}",
                                       name=f"m{half}")
                        nc.vector.tensor_tensor(
                            m_h[:], a_sb[:, k0:k0 + 2], n_sb[:, k0:k0 + 2],
                            OP.mult)
                        mh[half] = m_h
                        if not last:
                            nc.vector.tensor_tensor(
                                nhh[:, k0:k0 + 2], p_sb[:, k0:k0 + 2],
                                m_h[:], OP.add)
                    elif not last:
                        nc.vector.tensor_tensor(
                            nhh[:, k0:k0 + 2], a_sb[:, k0:k0 + 2],
                            n_sb[:, k0:k0 + 2], OP.mult)
                for half in (1, 0):
                    k0 = 2 * half
                    if s == 0:
                        nc.vector.tensor_tensor(
                            nh[:, k0:k0 + 2], a_sb[:, k0:k0 + 2],
                            n_sb[:, k0:k0 + 2], OP.mult)
                    else:
                        nc.vector.tensor_tensor(
                            nh[:, k0:k0 + 2], p_sb[:, k0:k0 + 2],
                            mh[half][:], OP.add)
                h16 = nh
                h8h_t = nhh

                # ---- x-part matmuls for s+1 ----
                if s + 1 < S:
                    pr = ps_r.tile([128, 4, BPC], FP32, tag="pr", name="pr")
                    pz = ps_z.tile([128, 4, BPC], FP32, tag="pz", name="pz")
                    pxn = ps_x.tile([128, 4, BPC], FP32, tag="pxn",
                                    name="pxn")
                    x_mms(cbuf[s + 1], [(pr, 0), (pxn, 2), (pz, 1)],
                          with_stop=False)
                    # stage the n-gate x psum to fp16 SBUF off the critical
                    # path (tn2 then runs in DVE fast mode)
                    x16n = xnp.tile([128, 4, BPC], FP16, tag="x16n",
                                    name="x16n")
                    nc.scalar.activation(x16n[:], pxn[:], AF.Copy)
                    del cbuf[s + 1]
                    if s + 5 < S:
                        cbuf[s + 5] = load_c(s + 5)

                # ---- interleaved scoring ----
                gisc = s // SGRP + 2
                q = s % SGRP
                if gisc <= NGRP - 1:
                    if q == 0 and gisc + 1 <= NGRP - 1:
                        load_feat(gisc + 1)
                    fc1_chunk(gisc, q)

            # ================= epilogue =================
            # transpose + copy + store per kt-half as the last chain's
            # halves land
            pt = ps_r.tile([128, 4, BPC], FP32, tag="pr", name="ptr")
            ot = ew.tile([128, H], FP32, tag="ot")
            for half in (1, 0):
                for kt in (2 * half, 2 * half + 1):
                    nc.tensor.transpose(pt[:, kt], h16[:, kt], idt[:])
                nc.scalar.activation(
                    ot[:, 256 * half:256 * half + 256],
                    pt[:, 2 * half:2 * half + 2].rearrange(
                        "p k b -> p (k b)"), AF.Copy)
                nc.sync.dma_start(
                    out.ap()[:, 256 * half:256 * half + 256],
                    ot[:, 256 * half:256 * half + 256])

    nc.compile()
    return nc


def _prep(C, Q, prev_M, fc1_w, fc2_w, W_ih, W_hh):
    """Host-side layout/dtype transforms + per-core sharding."""
    consts = {}
    consts["ident"] = np.eye(128, dtype=np.float32)
    # W_ih^T hi/lo fp8 in DR lhsT layout [p, pair, gate-tile, i, 128]
    hi, lo = _split8(np.ascontiguousarray(W_ih.T), 16.0)  # [K, G3]
    for nm, t in (("wih8h", hi), ("wih8l", lo)):
        consts[nm] = np.ascontiguousarray(
            t.reshape(2, 2, 128, NT, 128).transpose(2, 0, 3, 1, 4))
    # W_hh^T * 512 hi/lo fp8, same layout (h quantized at scale 1.0)
    hi, lo = _split8(np.ascontiguousarray(W_hh.T), 512.0)  # [H, G3]
    for nm, t in (("whh8h", hi), ("whh8l", lo)):
        consts[nm] = np.ascontiguousarray(
            t.reshape(2, 2, 128, NT, 128).transpose(2, 0, 3, 1, 4))
    f1p = np.zeros((128, 4 * H), np.float32)  # pad SH 120 -> 128
    f1p[:SH] = fc1_w
    hi, lo = _split8(np.ascontiguousarray(f1p.T), 16.0)  # [K, 128]
    for nm, t in (("f18h", hi), ("f18l", lo)):
        consts[nm] = np.ascontiguousarray(
            t.reshape(FK // 2, 2, 128, 128).transpose(2, 0, 1, 3))
    consts["f2t16"] = np.ascontiguousarray(fc2_w.T).astype(F16)

    in_maps = []
    for c in range(NCORES):
        lo_, hi_ = c * BPC, (c + 1) * BPC
        Cc = np.ascontiguousarray(C[lo_:hi_])          # [BPC, S, H]
        Qc = Q[lo_:hi_, 0]                              # [BPC, H]
        Mc = prev_M[lo_:hi_, 0]
        m = {}
        Ct = np.ascontiguousarray(Cc.transpose(1, 2, 0))   # [S, H, BPC]
        chi, clo = _split8(Ct, 32.0)  # [S, H, BPC]
        both = np.stack([chi, clo])   # [2, S, H, BPC]
        m["c8"] = np.ascontiguousarray(
            both.reshape(2, S, 2, 2, 128, BPC).transpose(0, 1, 4, 2, 3, 5))
        # feat [BPC, S, 4H]
        feat = np.concatenate(
            [Cc * Qc[:, None, :], Cc * Mc[:, None, :],
             np.abs(Cc - Qc[:, None, :]), np.abs(Cc - Mc[:, None, :])],
            axis=2)
        # -> [NGRP, 4H, SGRP, BPC]
        ftr = np.ascontiguousarray(
            feat.transpose(1, 2, 0).reshape(NGRP, SGRP, 4 * H, BPC)
            .transpose(0, 2, 1, 3))
        fhi, flo = _split8(ftr, 8.0)       # [NGRP, 4H, SGRP, BPC]
        both = np.stack([fhi, flo])
        m["feat8"] = np.ascontiguousarray(
            both.reshape(2, NGRP, FK // 2, 2, 128, SGRP * BPC)
            .transpose(0, 1, 4, 2, 3, 5))
        in_maps.append(m)
    return consts, in_maps


def kernel(C, Q, prev_M, fc1_w, fc1_b, fc2_w, fc2_b, W_ih, W_hh, b_ih, b_hh):
    from concourse.bass_utils import run_bass_kernel_spmd

    C = np.asarray(C, dtype=np.float32)
    Q = np.asarray(Q, dtype=np.float32)
    prev_M = np.asarray(prev_M, dtype=np.float32)
    fc1_w = np.asarray(fc1_w, np.float32)
    fc2_w = np.asarray(fc2_w, np.float32)
    W_ih = np.asarray(W_ih, np.float32)
    W_hh = np.asarray(W_hh, np.float32)
    fc1_b = np.asarray(fc1_b, np.float32)
    fc2_b = np.asarray(fc2_b, np.float32)
    b_ih = np.asarray(b_ih, np.float32)
    b_hh = np.asarray(b_hh, np.float32)
    assert not (np.any(fc1_b) or np.any(fc2_b) or np.any(b_ih)
                or np.any(b_hh)), "nonzero biases unsupported in v3"

    consts, in_maps = _prep(C, Q, prev_M, fc1_w, fc2_w, W_ih, W_hh)

    key = tuple(np.asarray(v).tobytes() for v in consts.values())
    kh = hash(key)
    if kh not in _CACHE:
        _CACHE[kh] = _build(consts)
    nc = _CACHE[kh]

    res = run_bass_kernel_spmd(nc, in_maps, list(range(NCORES)))
    h = np.concatenate([res.results[c]["out"] for c in range(NCORES)],
                       axis=0)
    return h[:, None, :].astype(np.float32)


# revision 38
# speedup vs baseline: 1.1601x; 1.0049x over previous
"""EpisodicMemory Trainium2 kernel, v3.

Data-parallel over batch across 8 NeuronCores (128 batch rows per core).

Layout is "flipped": the GRU state h and all gate pre-activations live as
[H-on-partitions (4 k-tiles of 128), batch-on-free(128)] tiles, so the
recurrent matmul h @ W_hh^T needs NO transposes: its rhs (moving operand)
is h itself, and the elementwise update produces h directly in that
layout. Per-sentence episodic gates are broadcast across partitions with a
stride-0 DMA from DRAM.

Precision plan (validated offline, rel err ~1.3e-2 vs 2e-2 budget):
 - x-part (C @ W_ih^T) and scoring fc1: 3-term error-compensated fp8-e4m3
   DoubleRow matmuls (hi/lo splits of both operands, Whi@Chi + Whi@Clo +
   Wlo@Chi), 0.5 cyc/row with K=256 per instruction.
 - h-part (h @ W_hh^T): fp8 DoubleRow, every gate 2-term WEIGHT-side
   compensated ((Whi+Wlo) @ hhi) so only the hi fp8 plane of h is ever
   needed. h stays in fp16 (quantized at scale 1.0, |h|<=1); the chain's
   p+m add is issued twice, fp8-out first (h8h) so the next step's h-part
   matmuls unblock before the fp16 adds complete.
 - tn/tn2 (r*hn + xn) are fp16 at the 512x psum scale; the n-gate x psum
   is pre-staged to fp16 SBUF (ACT copy, off the critical path) so tn2
   runs in the DVE 2x mode instead of reading psum at 1x.
 - All W matrices pre-scaled so every GRU psum holds 512x the preact
   (fc1: 128x); the 1/512 folds into the ACT sigmoid/tanh scale.
 - C / feat = [C*Q, C*prev_M, |C-Q|, |C-prev_M|] quantized host-side.

Per-step critical loop (the scan is latency-bound, not engine-bound):
h8h -> r/n/z h-matmuls -> sigmoid(r) -> tn -> tn2 -> tanh (2 chunks) ->
m -> h8h. DVE work is kept just under the loop period; a/t1/p fill the
tanh window. Startup: tiny consts DMA first, then fc1 weights + feat
group 0 in interleaved hi/lo quarters so scoring matmuls start ~4us in;
identity-tile warmup matmuls ramp the PE p-state during the DMA wait.
"""
import numpy as np
import ml_dtypes

H = 512
SH = 120
B = 1024
S = 64
NCORES = 8
BPC = B // NCORES  # 128
KH = H // 128      # 4
G3 = 3 * H
SGRP = 4
NGRP = S // SGRP   # 16
NT = G3 // 128     # 12 gate tiles
FK = 4 * H // 128  # 16 feat k-tiles
E4 = ml_dtypes.float8_e4m3
F16 = np.float16

N_WARMUP = 10      # identity matmuls to ramp the PE p-state during DMA wait

_CACHE = {}


def _q8(x):
    return np.clip(np.asarray(x, np.float32), -240.0, 240.0).astype(E4)


def _split8(x, scale):
    hi = _q8(x * scale)
    lo = _q8(x * scale - hi.astype(np.float32))
    return hi, lo


def _build(consts):
    import concourse.bass as bass
    import concourse.tile as tile
    from concourse import bacc, mybir

    FP32 = mybir.dt.float32
    FP16 = mybir.dt.float16
    FP8 = mybir.dt.float8e4
    OP = mybir.AluOpType
    AF = mybir.ActivationFunctionType
    PM = mybir.MatmulPerfMode

    nc = bacc.Bacc("TRN2", target_bir_lowering=False, debug=False,
                   num_devices=NCORES)

    # ---- external inputs (per core) ----
    c_t = nc.dram_tensor("c8", [2, S, 128, 2, 2, BPC], FP8,
                         kind="ExternalInput")  # [hi/lo, s, p, pair, i, b]
    f_t = nc.dram_tensor("feat8", [2, NGRP, 128, FK // 2, 2, SGRP * BPC],
                         FP8, kind="ExternalInput")
    out = nc.dram_tensor("out", [BPC, H], FP32, kind="ExternalOutput")

    # ---- inline consts ----
    dl = {}
    for k, v in consts.items():
        dl[k] = nc.inline_tensor(v, name=k)

    from contextlib import ExitStack
    with tile.TileContext(nc) as tc:
        with ExitStack() as ctx:
            cpool = ctx.enter_context(tc.tile_pool(name="const", bufs=1))
            cpool2 = ctx.enter_context(tc.tile_pool(name="const2", bufs=1))
            cstr = ctx.enter_context(tc.tile_pool(name="cstr", bufs=6))
            fstr = ctx.enter_context(tc.tile_pool(name="fstr", bufs=3))
            hpool = ctx.enter_context(tc.tile_pool(name="h", bufs=3))
            hq = ctx.enter_context(tc.tile_pool(name="hq", bufs=4))
            gpool = ctx.enter_context(tc.tile_pool(name="g", bufs=4))
            gdram = ctx.enter_context(tc.tile_pool(name="gd", bufs=4,
                                                   space="DRAM"))
            ew = ctx.enter_context(tc.tile_pool(name="ew", bufs=3))
            ew2 = ctx.enter_context(tc.tile_pool(name="ew2", bufs=3))
            xnp = ctx.enter_context(tc.tile_pool(name="xn16", bufs=3))
            sco = ctx.enter_context(tc.tile_pool(name="sco", bufs=2))
            ps_r = ctx.enter_context(tc.tile_pool(name="ps_r", bufs=2,
                                                  space="PSUM"))
            ps_z = ctx.enter_context(tc.tile_pool(name="ps_z", bufs=2,
                                                  space="PSUM"))
            ps_x = ctx.enter_context(tc.tile_pool(name="ps_x", bufs=1,
                                                  space="PSUM"))
            ps_h = ctx.enter_context(tc.tile_pool(name="ps_h", bufs=1,
                                                  space="PSUM"))
            ps_f = ctx.enter_context(tc.tile_pool(name="ps_f", bufs=2,
                                                  space="PSUM"))

            # ---- tiny consts first: identity (warmup + epilogue), fc2 ----
            idt = cpool.tile([128, 128], FP32, tag="idt")
            nc.sync.dma_start(idt[:], dl["ident"].ap())
            f2t = cpool2.tile([SH, 1], FP16, tag="f2t")
            nc.sync.dma_start(f2t[:], dl["f2t16"].ap())
            # touch every activation function once so the ACT table loads
            # happen during the const-DMA wait, not on the scan chain
            warm = cpool2.tile([1, 4], FP32, tag="warm")
            for af in (AF.Sigmoid, AF.Tanh, AF.Copy):
                nc.scalar.activation(warm[:], idt[0:1, 0:4], af)
            # PE p-state warmup: fp32 identity matmuls (512 cyc each), no
            # data deps beyond idt, so they run while the big DMAs stream.
            # Aliases the fc1 psum tag: PE is in-order, so no extra blocking.
            wps = ps_f.tile([128, SGRP * BPC], FP32, tag="pps", name="wps")
            for i in range(N_WARMUP):
                nc.tensor.matmul(wps[:, 0:128], idt[:], idt[:],
                                 start=(i == 0), stop=(i == N_WARMUP - 1))

            # ---- fc1 weights, then feat g0 (quartered) so scoring starts
            # as early as possible ----
            f1h = cpool2.tile([128, FK // 2, 2, 128], FP8, tag="f1h")
            nc.sync.dma_start(f1h[:], dl["f18h"].ap())
            f1l = cpool2.tile([128, FK // 2, 2, 128], FP8, tag="f1l")
            nc.sync.dma_start(f1l[:], dl["f18l"].ap())

            # ================= helpers =================
            def load_c(s):
                ch = cstr.tile([128, 2, 2, BPC], FP8, tag="csh")
                nc.sync.dma_start(ch[:], c_t.ap()[0, s])
                cl = cstr.tile([128, 2, 2, BPC], FP8, tag="csl")
                nc.sync.dma_start(cl[:], c_t.ap()[1, s])
                return (ch, cl)

            def x_mms(cts, banks, with_stop):
                """x-part matmuls for one sentence into the given
                [(psum, gate)] banks. pr/pz use per-jj-half accumulation
                regions (the chain's sigmoid chunks read each half as soon
                as the h-part closes it); pxn is a single region, closed
                here (the h-part never accumulates into it)."""
                ch, cl = cts
                for pb, gate in banks:
                    for jj in range(4):
                        gt_ = gate * 4 + jj
                        for pair in range(2):
                            terms = [(wih_h, ch), (wih_h, cl),
                                     (wih_l, ch)]
                            for ti, (wt_, ct_) in enumerate(terms):
                                nc.tensor.matmul(
                                    pb[:, jj],
                                    wt_[:, pair, gt_],
                                    ct_[:, pair],
                                    start=(jj == 0 and pair == 0
                                           and ti == 0),
                                    stop=((with_stop or gate == 2)
                                          and jj == 3
                                          and pair == 1 and ti == 2),
                                    perf_mode=PM.DoubleRow)

            def h_mms(h8h_t, pr, pz, pxn, phn):
                """h-part fp8 DR matmuls: every gate is 2-term weight-side
                compensated ((Whi+Wlo) @ hhi), so only the hi plane of h is
                needed — the chain never produces a lo plane and the n-gate
                psum closes right after the hi plane lands. Gate order
                r, n, z: r feeds the sigmoid, n closes phn for tn, z last."""
                for gate, pb in ((0, pr), (2, phn), (1, pz)):
                    for ti, wt_ in enumerate([whh_h, whh_l]):
                        for pair in (1, 0):
                            for jj in range(4):
                                nc.tensor.matmul(
                                    pb[:, jj], wt_[:, pair, 4 * gate + jj],
                                    h8h_t[:, 2 * pair:2 * pair + 2],
                                    start=(gate == 2 and ti == 0
                                           and pair == 1 and jj == 0),
                                    stop=(ti == 1 and pair == 0
                                          and jj == 3),
                                    perf_mode=PM.DoubleRow)

            # ---- scoring machinery ----
            grp = {}

            def load_feat(gi, quarters=False):
                # split loads along k so downstream fc1 chunks unblock
                # progressively; interleave hi/lo quarters for the startup
                # groups so chunk q unblocks after 1/4 of the bytes
                fh = fstr.tile([128, FK // 2, 2, SGRP * BPC], FP8,
                               tag="feath")
                fl = fstr.tile([128, FK // 2, 2, SGRP * BPC], FP8,
                               tag="featl")
                if quarters:
                    for q0 in range(0, 8, 2):
                        nc.sync.dma_start(fh[:, q0:q0 + 2],
                                          f_t.ap()[0, gi, :, q0:q0 + 2])
                        nc.sync.dma_start(fl[:, q0:q0 + 2],
                                          f_t.ap()[1, gi, :, q0:q0 + 2])
                else:
                    nc.sync.dma_start(fh[:, 0:4], f_t.ap()[0, gi, :, 0:4])
                    nc.sync.dma_start(fh[:, 4:8], f_t.ap()[0, gi, :, 4:8])
                    nc.sync.dma_start(fl[:, 0:4], f_t.ap()[1, gi, :, 0:4])
                    nc.sync.dma_start(fl[:, 4:8], f_t.ap()[1, gi, :, 4:8])
                grp[gi] = {"feat": (fh, fl)}

            def fc1_chunk(gi, q):
                """Quarter q of group gi's fc1 matmuls."""
                st = grp[gi]
                if q == 0:
                    st["pps"] = ps_f.tile([128, SGRP * BPC], FP32, tag="pps",
                                          name="pps")
                pps = st["pps"]
                fh, fl = st["feat"]
                for pair in range(2 * q, 2 * q + 2):
                    terms = [(f1h, fh), (f1h, fl), (f1l, fh)]
                    for ti, (wt_, ft_) in enumerate(terms):
                        nc.tensor.matmul(
                            pps[:], wt_[:, pair], ft_[:, pair],
                            start=(pair == 0 and ti == 0),
                            stop=(pair == FK // 2 - 1 and ti == 2),
                            perf_mode=PM.DoubleRow)
                if q == 3:
                    finish_group(gi)

            def finish_group(gi):
                st = grp[gi]
                pps = st["pps"]
                h1 = sco.tile([SH, SGRP * BPC], FP16, tag="h1")
                nc.scalar.activation(h1[:], pps[0:SH, :], AF.Tanh,
                                     scale=1.0 / 128)
                nc.tensor.matmul(pps[0:1, :], f2t[:], h1[:],
                                 start=True, stop=True)
                gt = gpool.tile([1, SGRP * BPC], FP16, tag="gt")
                nc.scalar.activation(gt[:], pps[0:1, :], AF.Sigmoid)
                gd = gdram.tile([1, SGRP * BPC], FP16, tag="gd")
                nc.sync.dma_start(gd[:], gt[:])
                grep = gpool.tile([128, SGRP, BPC], FP16, tag="grep")
                nc.sync.dma_start(
                    grep[:], gd[:].broadcast_to([128, SGRP * BPC]))
                st["grep"] = grep
                del st["pps"], st["feat"]

            # ================= prologue =================
            load_feat(0, quarters=True)
            for q in range(4):
                fc1_chunk(0, q)

            wih_h = cpool.tile([128, 2, NT, 2, 128], FP8, tag="wih_h")
            nc.sync.dma_start(wih_h[:], dl["wih8h"].ap())
            wih_l = cpool.tile([128, 2, NT, 2, 128], FP8, tag="wih_l")
            nc.sync.dma_start(wih_l[:], dl["wih8l"].ap())
            cbuf = {0: load_c(0)}
            pz = ps_z.tile([128, 4, BPC], FP32, tag="pz", name="pz")
            pxn = ps_x.tile([128, 4, BPC], FP32, tag="pxn", name="pxn")
            x_mms(cbuf[0], [(pxn, 2), (pz, 1)], with_stop=True)
            del cbuf[0]

            # recurrent weights right behind c0 (step 1's h-part gates on
            # whh_h); group 1's scoring runs inside scan steps 0-3
            whh_h = cpool.tile([128, 2, NT, 2, 128], FP8, tag="whh_h")
            nc.sync.dma_start(whh_h[:], dl["whh8h"].ap())
            whh_l = cpool.tile([128, 2, NT, 2, 128], FP8, tag="whh_l")
            nc.sync.dma_start(whh_l[:], dl["whh8l"].ap())

            for s in range(1, 5):
                cbuf[s] = load_c(s)
            load_feat(1, quarters=True)
            load_feat(2)

            h16 = None
            h8h_t = None
            pr = None
            ISC = 1.0 / 512

            # ================= scan =================
            for s in range(S):
                # ---- h-part matmuls (s>0) ----
                if s > 0:
                    phn = ps_h.tile([128, 4, BPC], FP32, tag="phn",
                                    name="phn")
                    h_mms(h8h_t, pr, pz, pxn, phn)

                # ---- elementwise chain ----
                gi = s // SGRP
                j = s % SGRP
                grep = grp[gi]["grep"]

                if s > 0:
                    r_sb = ew.tile([128, 4, BPC], FP16, tag="r")
                    nc.scalar.activation(r_sb[:], pr[:], AF.Sigmoid,
                                         scale=ISC)
                    # tn/tn2 in fp16 at the 512x psum scale: tn2's second
                    # operand is the pre-copied fp16 xn, so the add runs in
                    # DVE fast mode instead of reading psum at 1x
                    tn = ew2.tile([128, 4, BPC], FP16, tag="tn")
                    nc.vector.tensor_tensor(tn[:], r_sb[:], phn[:], OP.mult)
                w_sb = ew.tile([128, 4, BPC], FP16, tag="w")
                nc.scalar.activation(w_sb[:], pz[:], AF.Sigmoid, scale=-ISC)
                if s > 0:
                    tn2 = ew2.tile([128, 4, BPC], FP16, tag="tn2")
                    nc.vector.tensor_tensor(tn2[:], tn[:], x16n[:], OP.add)
                a_sb = ew.tile([128, 4, BPC], FP16, tag="a")
                nc.vector.tensor_tensor(
                    a_sb[:], w_sb[:],
                    grep[:, j].unsqueeze(1).broadcast_to([128, 4, BPC]),
                    OP.mult)
                # a / t1 / p are off the critical chain (they fill the DVE
                # while tanh runs on ACT)
                if s > 0:
                    t1 = ew2.tile([128, 4, BPC], FP16, tag="t1")
                    nc.vector.tensor_tensor(t1[:], a_sb[:], h16[:], OP.mult)
                    p_sb = ew2.tile([128, 4, BPC], FP16, tag="p")
                    nc.vector.tensor_tensor(p_sb[:], h16[:], t1[:],
                                            OP.subtract)
                n_sb = ew.tile([128, 4, BPC], FP16, tag="n")
                CHT = ((2, 4), (0, 2))
                for c0_, c1_ in CHT:
                    if s > 0:
                        nc.scalar.activation(n_sb[:, c0_:c1_],
                                             tn2[:, c0_:c1_], AF.Tanh,
                                             scale=ISC)
                    else:
                        nc.scalar.activation(n_sb[:, c0_:c1_],
                                             pxn[:, c0_:c1_], AF.Tanh,
                                             scale=ISC)
                last = s == S - 1
                if last:
                    nh = ew.tile([128, KH, BPC], FP32, tag="hf")
                    nhh = None
                else:
                    nh = hpool.tile([128, KH, BPC], FP16, tag="h", name="h")
                    nhh = hq.tile([128, KH, BPC], FP8, tag="h8h",
                                  name="h8h")
                # post-tanh tail, all on DVE (no cross-engine hops): the fp8
                # hi-plane adds come FIRST — they alone unblock ALL of next
                # step's h-part matmuls; the fp16 adds trail
                mh = {}
                for half in (1, 0):
                    k0 = 2 * half
                    if s > 0:
                        m_h = ew2.tile([128, 2, BPC], FP16, tag=f"m{half}",
                                       name=f"m{half}")
                        nc.vector.tensor_tensor(
                            m_h[:], a_sb[:, k0:k0 + 2], n_sb[:, k0:k0 + 2],
                            OP.mult)
                        mh[half] = m_h
                        if not last:
                            nc.vector.tensor_tensor(
                                nhh[:, k0:k0 + 2], p_sb[:, k0:k0 + 2],
                                m_h[:], OP.add)
                    elif not last:
                        nc.vector.tensor_tensor(
                            nhh[:, k0:k0 + 2], a_sb[:, k0:k0 + 2],
                            n_sb[:, k0:k0 + 2], OP.mult)
                for half in (1, 0):
                    k0 = 2 * half
                    if s == 0:
                        nc.vector.tensor_tensor(
                            nh[:, k0:k0 + 2], a_sb[:, k0:k0 + 2],
                            n_sb[:, k0:k0 + 2], OP.mult)
                    else:
                        nc.vector.tensor_tensor(
                            nh[:, k0:k0 + 2], p_sb[:, k0:k0 + 2],
                            mh[half][:], OP.add)
                h16 = nh
                h8h_t = nhh

                # ---- x-part matmuls for s+1 ----
                if s + 1 < S:
                    pr = ps_r.tile([128, 4, BPC], FP32, tag="pr", name="pr")
                    pz = ps_z.tile([128, 4, BPC], FP32, tag="pz", name="pz")
                    pxn = ps_x.tile([128, 4, BPC], FP32, tag="pxn",
                                    name="pxn")
                    x_mms(cbuf[s + 1], [(pr, 0), (pxn, 2), (pz, 1)],
                          with_stop=False)
                    # stage the n-gate x psum to fp16 SBUF off the critical
                    # path (tn2 then runs in DVE fast mode)
                    x16n = xnp.tile([128, 4, BPC], FP16, tag="x16n",
                                    name="x16n")
                    nc.scalar.activation(x16n[:], pxn[:], AF.Copy)
                    del cbuf[s + 1]
                    if s + 5 < S:
                        cbuf[s + 5] = load_c(s + 5)

                # ---- interleaved scoring ----
                # group 1 (needed from s=4) is scored during steps 0-3
                if s < 4:
                    fc1_chunk(1, s)
                gisc = s // SGRP + 2
                q = s % SGRP
                if gisc <= NGRP - 1:
                    if q == 0 and gisc + 1 <= NGRP - 1:
                        load_feat(gisc + 1)
                    fc1_chunk(gisc, q)

            # ================= epilogue =================
            # transpose + copy + store per kt-half as the last chain's
            # halves land
            pt = ps_r.tile([128, 4, BPC], FP32, tag="pr", name="ptr")
            ot = ew.tile([128, H], FP32, tag="ot")
            for half in (1, 0):
                for kt in (2 * half, 2 * half + 1):
                    nc.tensor.transpose(pt[:, kt], h16[:, kt], idt[:])
                nc.scalar.activation(
                    ot[:, 256 * half:256 * half + 256],
                    pt[:, 2 * half:2 * half + 2].rearrange(
                        "p k b -> p (k b)"), AF.Copy)
                nc.sync.dma_start(
                    out.ap()[:, 256 * half:256 * half + 256],
                    ot[:, 256 * half:256 * half + 256])

    nc.compile()
    return nc


def _prep(C, Q, prev_M, fc1_w, fc2_w, W_ih, W_hh):
    """Host-side layout/dtype transforms + per-core sharding."""
    consts = {}
    consts["ident"] = np.eye(128, dtype=np.float32)
    # W_ih^T hi/lo fp8 in DR lhsT layout [p, pair, gate-tile, i, 128]
    hi, lo = _split8(np.ascontiguousarray(W_ih.T), 16.0)  # [K, G3]
    for nm, t in (("wih8h", hi), ("wih8l", lo)):
        consts[nm] = np.ascontiguousarray(
            t.reshape(2, 2, 128, NT, 128).transpose(2, 0, 3, 1, 4))
    # W_hh^T * 512 hi/lo fp8, same layout (h quantized at scale 1.0)
    hi, lo = _split8(np.ascontiguousarray(W_hh.T), 512.0)  # [H, G3]
    for nm, t in (("whh8h", hi), ("whh8l", lo)):
        consts[nm] = np.ascontiguousarray(
            t.reshape(2, 2, 128, NT, 128).transpose(2, 0, 3, 1, 4))
    f1p = np.zeros((128, 4 * H), np.float32)  # pad SH 120 -> 128
    f1p[:SH] = fc1_w
    hi, lo = _split8(np.ascontiguousarray(f1p.T), 16.0)  # [K, 128]
    for nm, t in (("f18h", hi), ("f18l", lo)):
        consts[nm] = np.ascontiguousarray(
            t.reshape(FK // 2, 2, 128, 128).transpose(2, 0, 1, 3))
    consts["f2t16"] = np.ascontiguousarray(fc2_w.T).astype(F16)

    in_maps = []
    for c in range(NCORES):
        lo_, hi_ = c * BPC, (c + 1) * BPC
        Cc = np.ascontiguousarray(C[lo_:hi_])          # [BPC, S, H]
        Qc = Q[lo_:hi_, 0]                              # [BPC, H]
        Mc = prev_M[lo_:hi_, 0]
        m = {}
        Ct = np.ascontiguousarray(Cc.transpose(1, 2, 0))   # [S, H, BPC]
        chi, clo = _split8(Ct, 32.0)  # [S, H, BPC]
        both = np.stack([chi, clo])   # [2, S, H, BPC]
        m["c8"] = np.ascontiguousarray(
            both.reshape(2, S, 2, 2, 128, BPC).transpose(0, 1, 4, 2, 3, 5))
        # feat [BPC, S, 4H]
        feat = np.concatenate(
            [Cc * Qc[:, None, :], Cc * Mc[:, None, :],
             np.abs(Cc - Qc[:, None, :]), np.abs(Cc - Mc[:, None, :])],
            axis=2)
        # -> [NGRP, 4H, SGRP, BPC]
        ftr = np.ascontiguousarray(
            feat.transpose(1, 2, 0).reshape(NGRP, SGRP, 4 * H, BPC)
            .transpose(0, 2, 1, 3))
        fhi, flo = _split8(ftr, 8.0)       # [NGRP, 4H, SGRP, BPC]
        both = np.stack([fhi, flo])
        m["feat8"] = np.ascontiguousarray(
            both.reshape(2, NGRP, FK // 2, 2, 128, SGRP * BPC)
            .transpose(0, 1, 4, 2, 3, 5))
        in_maps.append(m)
    return consts, in_maps


def kernel(C, Q, prev_M, fc1_w, fc1_b, fc2_w, fc2_b, W_ih, W_hh, b_ih, b_hh):
    from concourse.bass_utils import run_bass_kernel_spmd

    C = np.asarray(C, dtype=np.float32)
    Q = np.asarray(Q, dtype=np.float32)
    prev_M = np.asarray(prev_M, dtype=np.float32)
    fc1_w = np.asarray(fc1_w, np.float32)
    fc2_w = np.asarray(fc2_w, np.float32)
    W_ih = np.asarray(W_ih, np.float32)
    W_hh = np.asarray(W_hh, np.float32)
    fc1_b = np.asarray(fc1_b, np.float32)
    fc2_b = np.asarray(fc2_b, np.float32)
    b_ih = np.asarray(b_ih, np.float32)
    b_hh = np.asarray(b_hh, np.float32)
    assert not (np.any(fc1_b) or np.any(fc2_b) or np.any(b_ih)
                or np.any(b_hh)), "nonzero biases unsupported in v3"

    consts, in_maps = _prep(C, Q, prev_M, fc1_w, fc2_w, W_ih, W_hh)

    key = tuple(np.asarray(v).tobytes() for v in consts.values())
    kh = hash(key)
    if kh not in _CACHE:
        _CACHE[kh] = _build(consts)
    nc = _CACHE[kh]

    res = run_bass_kernel_spmd(nc, in_maps, list(range(NCORES)))
    h = np.concatenate([res.results[c]["out"] for c in range(NCORES)],
                       axis=0)
    return h[:, None, :].astype(np.float32)


# revision 39
# speedup vs baseline: 1.2053x; 1.0389x over previous
"""EpisodicMemory Trainium2 kernel, v3.

Data-parallel over batch across 8 NeuronCores (128 batch rows per core).

Layout is "flipped": the GRU state h and all gate pre-activations live as
[H-on-partitions (4 k-tiles of 128), batch-on-free(128)] tiles, so the
recurrent matmul h @ W_hh^T needs NO transposes: its rhs (moving operand)
is h itself, and the elementwise update produces h directly in that
layout. Per-sentence episodic gates are broadcast across partitions with a
stride-0 DMA from DRAM.

Precision plan (validated offline, rel err ~1.3e-2 vs 2e-2 budget):
 - x-part (C @ W_ih^T) and scoring fc1: 3-term error-compensated fp8-e4m3
   DoubleRow matmuls (hi/lo splits of both operands, Whi@Chi + Whi@Clo +
   Wlo@Chi), 0.5 cyc/row with K=256 per instruction.
 - h-part (h @ W_hh^T): fp8 DoubleRow, every gate 2-term WEIGHT-side
   compensated ((Whi+Wlo) @ hhi) so only the hi fp8 plane of h is ever
   needed. h stays in fp16 (quantized at scale 1.0, |h|<=1); the chain's
   p+m add is issued twice, fp8-out first (h8h) so the next step's h-part
   matmuls unblock before the fp16 adds complete.
 - tn/tn2 (r*hn + xn) are fp16 at the 512x psum scale; the n-gate x psum
   is pre-staged to fp16 SBUF (ACT copy, off the critical path) so tn2
   runs in the DVE 2x mode instead of reading psum at 1x.
 - All W matrices pre-scaled so every GRU psum holds 512x the preact
   (fc1: 128x); the 1/512 folds into the ACT sigmoid/tanh scale.
 - C / feat = [C*Q, C*prev_M, |C-Q|, |C-prev_M|] quantized host-side.

Per-step critical loop (the scan is latency-bound, not engine-bound):
h8h -> r/n/z h-matmuls -> sigmoid(r) -> tn -> tn2 -> tanh (2 chunks) ->
m -> h8h. DVE work is kept just under the loop period; a/t1/p fill the
tanh window. Startup: tiny consts DMA first, then fc1 weights + feat
group 0 in interleaved hi/lo quarters so scoring matmuls start ~4us in;
identity-tile warmup matmuls ramp the PE p-state during the DMA wait.
"""
import numpy as np
import ml_dtypes

H = 512
SH = 120
B = 1024
S = 64
NCORES = 8
BPC = B // NCORES  # 128
KH = H // 128      # 4
G3 = 3 * H
SGRP = 4
NGRP = S // SGRP   # 16
NT = G3 // 128     # 12 gate tiles
FK = 4 * H // 128  # 16 feat k-tiles
E4 = ml_dtypes.float8_e4m3
F16 = np.float16

N_WARMUP = 10      # identity matmuls to ramp the PE p-state during DMA wait

_CACHE = {}


def _q8(x):
    return np.clip(np.asarray(x, np.float32), -240.0, 240.0).astype(E4)


def _split8(x, scale):
    hi = _q8(x * scale)
    lo = _q8(x * scale - hi.astype(np.float32))
    return hi, lo


def _build(consts):
    import concourse.bass as bass
    import concourse.tile as tile
    from concourse import bacc, mybir

    FP32 = mybir.dt.float32
    FP16 = mybir.dt.float16
    FP8 = mybir.dt.float8e4
    OP = mybir.AluOpType
    AF = mybir.ActivationFunctionType
    PM = mybir.MatmulPerfMode

    nc = bacc.Bacc("TRN2", target_bir_lowering=False, debug=False,
                   num_devices=NCORES)

    # ---- external inputs (per core) ----
    c_t = nc.dram_tensor("c8", [2, S, 128, 2, 2, BPC], FP8,
                         kind="ExternalInput")  # [hi/lo, s, p, pair, i, b]
    f_t = nc.dram_tensor("feat8", [2, NGRP, 128, FK // 2, 2, SGRP * BPC],
                         FP8, kind="ExternalInput")
    out = nc.dram_tensor("out", [BPC, H], FP32, kind="ExternalOutput")

    # ---- inline consts ----
    dl = {}
    for k, v in consts.items():
        dl[k] = nc.inline_tensor(v, name=k)

    from contextlib import ExitStack
    with tile.TileContext(nc) as tc:
        with ExitStack() as ctx:
            cpool = ctx.enter_context(tc.tile_pool(name="const", bufs=1))
            cpool2 = ctx.enter_context(tc.tile_pool(name="const2", bufs=1))
            cstr = ctx.enter_context(tc.tile_pool(name="cstr", bufs=6))
            fstr = ctx.enter_context(tc.tile_pool(name="fstr", bufs=3))
            hpool = ctx.enter_context(tc.tile_pool(name="h", bufs=3))
            hq = ctx.enter_context(tc.tile_pool(name="hq", bufs=4))
            gpool = ctx.enter_context(tc.tile_pool(name="g", bufs=4))
            gdram = ctx.enter_context(tc.tile_pool(name="gd", bufs=4,
                                                   space="DRAM"))
            ew = ctx.enter_context(tc.tile_pool(name="ew", bufs=3))
            ew2 = ctx.enter_context(tc.tile_pool(name="ew2", bufs=3))
            xnp = ctx.enter_context(tc.tile_pool(name="xn16", bufs=3))
            sco = ctx.enter_context(tc.tile_pool(name="sco", bufs=2))
            ps_r = ctx.enter_context(tc.tile_pool(name="ps_r", bufs=2,
                                                  space="PSUM"))
            ps_z = ctx.enter_context(tc.tile_pool(name="ps_z", bufs=2,
                                                  space="PSUM"))
            ps_x = ctx.enter_context(tc.tile_pool(name="ps_x", bufs=1,
                                                  space="PSUM"))
            ps_h = ctx.enter_context(tc.tile_pool(name="ps_h", bufs=1,
                                                  space="PSUM"))
            ps_f = ctx.enter_context(tc.tile_pool(name="ps_f", bufs=2,
                                                  space="PSUM"))

            # ---- tiny consts first: identity (warmup + epilogue), fc2 ----
            idt = cpool.tile([128, 128], FP32, tag="idt")
            nc.sync.dma_start(idt[:], dl["ident"].ap())
            f2t = cpool2.tile([SH, 1], FP16, tag="f2t")
            nc.sync.dma_start(f2t[:], dl["f2t16"].ap())
            # touch every activation function once so the ACT table loads
            # happen during the const-DMA wait, not on the scan chain
            warm = cpool2.tile([1, 4], FP32, tag="warm")
            for af in (AF.Sigmoid, AF.Tanh, AF.Copy):
                nc.scalar.activation(warm[:], idt[0:1, 0:4], af)
            # PE p-state warmup: fp32 identity matmuls (512 cyc each), no
            # data deps beyond idt, so they run while the big DMAs stream.
            # Aliases the fc1 psum tag: PE is in-order, so no extra blocking.
            wps = ps_f.tile([128, SGRP * BPC], FP32, tag="pps", name="wps")
            for i in range(N_WARMUP):
                nc.tensor.matmul(wps[:, 0:128], idt[:], idt[:],
                                 start=(i == 0), stop=(i == N_WARMUP - 1))

            # ---- fc1 weights, then feat g0 (quartered) so scoring starts
            # as early as possible ----
            f1h = cpool2.tile([128, FK // 2, 2, 128], FP8, tag="f1h")
            nc.sync.dma_start(f1h[:], dl["f18h"].ap())
            f1l = cpool2.tile([128, FK // 2, 2, 128], FP8, tag="f1l")
            nc.sync.dma_start(f1l[:], dl["f18l"].ap())

            # ================= helpers =================
            def load_c(s):
                ch = cstr.tile([128, 2, 2, BPC], FP8, tag="csh")
                nc.sync.dma_start(ch[:], c_t.ap()[0, s])
                cl = cstr.tile([128, 2, 2, BPC], FP8, tag="csl")
                nc.sync.dma_start(cl[:], c_t.ap()[1, s])
                return (ch, cl)

            def x_mms(cts, banks, with_stop):
                """x-part matmuls for one sentence into the given
                [(psum, gate)] banks. pr/pz use per-jj-half accumulation
                regions (the chain's sigmoid chunks read each half as soon
                as the h-part closes it); pxn is a single region, closed
                here (the h-part never accumulates into it)."""
                ch, cl = cts
                for pb, gate in banks:
                    for jj in range(4):
                        gt_ = gate * 4 + jj
                        for pair in range(2):
                            terms = [(wih_h, ch), (wih_h, cl),
                                     (wih_l, ch)]
                            for ti, (wt_, ct_) in enumerate(terms):
                                nc.tensor.matmul(
                                    pb[:, jj],
                                    wt_[:, pair, gt_],
                                    ct_[:, pair],
                                    start=(jj == 0 and pair == 0
                                           and ti == 0),
                                    stop=((with_stop or gate == 2)
                                          and jj == 3
                                          and pair == 1 and ti == 2),
                                    perf_mode=PM.DoubleRow)

            def h_mms(h8h_t, pr, pz, pxn, phn):
                """h-part fp8 DR matmuls: every gate is 2-term weight-side
                compensated ((Whi+Wlo) @ hhi), so only the hi plane of h is
                needed — the chain never produces a lo plane and the n-gate
                psum closes right after the hi plane lands. Gate order
                r, n, z: r feeds the sigmoid, n closes phn for tn, z last."""
                for gate, pb in ((0, pr), (2, phn), (1, pz)):
                    # r gate: 1 term (Whi @ hhi) — halving its matmul block
                    # lets sigmoid(r), which is on the serial loop, start
                    # ~200ns earlier. z/n: 2-term weight-side.
                    terms = [whh_h] if gate == 0 else [whh_h, whh_l]
                    for ti, wt_ in enumerate(terms):
                        for pair in (1, 0):
                            for jj in range(4):
                                nc.tensor.matmul(
                                    pb[:, jj], wt_[:, pair, 4 * gate + jj],
                                    h8h_t[:, 2 * pair:2 * pair + 2],
                                    start=(gate == 2 and ti == 0
                                           and pair == 1 and jj == 0),
                                    stop=(ti == len(terms) - 1
                                          and pair == 0 and jj == 3),
                                    perf_mode=PM.DoubleRow)

            # ---- scoring machinery ----
            grp = {}

            def load_feat(gi, quarters=False):
                # split loads along k so downstream fc1 chunks unblock
                # progressively; interleave hi/lo quarters for the startup
                # groups so chunk q unblocks after 1/4 of the bytes
                fh = fstr.tile([128, FK // 2, 2, SGRP * BPC], FP8,
                               tag="feath")
                fl = fstr.tile([128, FK // 2, 2, SGRP * BPC], FP8,
                               tag="featl")
                if quarters:
                    for q0 in range(0, 8, 2):
                        nc.sync.dma_start(fh[:, q0:q0 + 2],
                                          f_t.ap()[0, gi, :, q0:q0 + 2])
                        nc.sync.dma_start(fl[:, q0:q0 + 2],
                                          f_t.ap()[1, gi, :, q0:q0 + 2])
                else:
                    nc.sync.dma_start(fh[:, 0:4], f_t.ap()[0, gi, :, 0:4])
                    nc.sync.dma_start(fh[:, 4:8], f_t.ap()[0, gi, :, 4:8])
                    nc.sync.dma_start(fl[:, 0:4], f_t.ap()[1, gi, :, 0:4])
                    nc.sync.dma_start(fl[:, 4:8], f_t.ap()[1, gi, :, 4:8])
                grp[gi] = {"feat": (fh, fl)}

            def fc1_chunk(gi, q):
                """Quarter q of group gi's fc1 matmuls."""
                st = grp[gi]
                if q == 0:
                    st["pps"] = ps_f.tile([128, SGRP * BPC], FP32, tag="pps",
                                          name="pps")
                pps = st["pps"]
                fh, fl = st["feat"]
                for pair in range(2 * q, 2 * q + 2):
                    terms = [(f1h, fh), (f1h, fl), (f1l, fh)]
                    for ti, (wt_, ft_) in enumerate(terms):
                        nc.tensor.matmul(
                            pps[:], wt_[:, pair], ft_[:, pair],
                            start=(pair == 0 and ti == 0),
                            stop=(pair == FK // 2 - 1 and ti == 2),
                            perf_mode=PM.DoubleRow)
                if q == 3:
                    finish_group(gi)

            def finish_group(gi):
                st = grp[gi]
                pps = st["pps"]
                h1 = sco.tile([SH, SGRP * BPC], FP16, tag="h1")
                nc.scalar.activation(h1[:], pps[0:SH, :], AF.Tanh,
                                     scale=1.0 / 128)
                nc.tensor.matmul(pps[0:1, :], f2t[:], h1[:],
                                 start=True, stop=True)
                gt = gpool.tile([1, SGRP * BPC], FP16, tag="gt")
                nc.scalar.activation(gt[:], pps[0:1, :], AF.Sigmoid)
                gd = gdram.tile([1, SGRP * BPC], FP16, tag="gd")
                nc.sync.dma_start(gd[:], gt[:])
                grep = gpool.tile([128, SGRP, BPC], FP16, tag="grep")
                nc.sync.dma_start(
                    grep[:], gd[:].broadcast_to([128, SGRP * BPC]))
                st["grep"] = grep
                del st["pps"], st["feat"]

            # ================= prologue =================
            load_feat(0, quarters=True)
            for q in range(4):
                fc1_chunk(0, q)

            wih_h = cpool.tile([128, 2, NT, 2, 128], FP8, tag="wih_h")
            nc.sync.dma_start(wih_h[:], dl["wih8h"].ap())
            wih_l = cpool.tile([128, 2, NT, 2, 128], FP8, tag="wih_l")
            nc.sync.dma_start(wih_l[:], dl["wih8l"].ap())
            cbuf = {0: load_c(0)}
            pz = ps_z.tile([128, 4, BPC], FP32, tag="pz", name="pz")
            pxn = ps_x.tile([128, 4, BPC], FP32, tag="pxn", name="pxn")
            x_mms(cbuf[0], [(pxn, 2), (pz, 1)], with_stop=True)
            del cbuf[0]

            # recurrent weights right behind c0 (step 1's h-part gates on
            # whh_h); group 1's scoring runs inside scan steps 0-3
            whh_h = cpool.tile([128, 2, NT, 2, 128], FP8, tag="whh_h")
            nc.sync.dma_start(whh_h[:], dl["whh8h"].ap())
            whh_l = cpool.tile([128, 2, NT, 2, 128], FP8, tag="whh_l")
            nc.sync.dma_start(whh_l[:], dl["whh8l"].ap())

            for s in range(1, 5):
                cbuf[s] = load_c(s)
            load_feat(1, quarters=True)
            load_feat(2)

            h16 = None
            h8h_t = None
            pr = None
            ISC = 1.0 / 512

            # ================= scan =================
            for s in range(S):
                # ---- h-part matmuls (s>0) ----
                if s > 0:
                    phn = ps_h.tile([128, 4, BPC], FP32, tag="phn",
                                    name="phn")
                    h_mms(h8h_t, pr, pz, pxn, phn)

                # ---- elementwise chain ----
                gi = s // SGRP
                j = s % SGRP
                grep = grp[gi]["grep"]

                if s > 0:
                    r_sb = ew.tile([128, 4, BPC], FP16, tag="r")
                    nc.scalar.activation(r_sb[:], pr[:], AF.Sigmoid,
                                         scale=ISC)
                    # tn/tn2 in fp16 at the 512x psum scale: tn2's second
                    # operand is the pre-copied fp16 xn, so the add runs in
                    # DVE fast mode instead of reading psum at 1x
                    tn = ew2.tile([128, 4, BPC], FP16, tag="tn")
                    nc.vector.tensor_tensor(tn[:], r_sb[:], phn[:], OP.mult)
                w_sb = ew.tile([128, 4, BPC], FP16, tag="w")
                nc.scalar.activation(w_sb[:], pz[:], AF.Sigmoid, scale=-ISC)
                if s > 0:
                    tn2 = ew2.tile([128, 4, BPC], FP16, tag="tn2")
                    nc.vector.tensor_tensor(tn2[:], tn[:], x16n[:], OP.add)
                a_sb = ew.tile([128, 4, BPC], FP16, tag="a")
                nc.vector.tensor_tensor(
                    a_sb[:], w_sb[:],
                    grep[:, j].unsqueeze(1).broadcast_to([128, 4, BPC]),
                    OP.mult)
                # a / t1 / p are off the critical chain (they fill the DVE
                # while tanh runs on ACT)
                if s > 0:
                    t1 = ew2.tile([128, 4, BPC], FP16, tag="t1")
                    nc.vector.tensor_tensor(t1[:], a_sb[:], h16[:], OP.mult)
                    p_sb = ew2.tile([128, 4, BPC], FP16, tag="p")
                    nc.vector.tensor_tensor(p_sb[:], h16[:], t1[:],
                                            OP.subtract)
                n_sb = ew.tile([128, 4, BPC], FP16, tag="n")
                CHT = ((2, 4), (0, 2))
                for c0_, c1_ in CHT:
                    if s > 0:
                        nc.scalar.activation(n_sb[:, c0_:c1_],
                                             tn2[:, c0_:c1_], AF.Tanh,
                                             scale=ISC)
                    else:
                        nc.scalar.activation(n_sb[:, c0_:c1_],
                                             pxn[:, c0_:c1_], AF.Tanh,
                                             scale=ISC)
                last = s == S - 1
                if last:
                    nh = ew.tile([128, KH, BPC], FP32, tag="hf")
                    nhh = None
                else:
                    nh = hpool.tile([128, KH, BPC], FP16, tag="h", name="h")
                    nhh = hq.tile([128, KH, BPC], FP8, tag="h8h",
                                  name="h8h")
                # post-tanh tail, all on DVE (no cross-engine hops): the fp8
                # hi-plane adds come FIRST — they alone unblock ALL of next
                # step's h-part matmuls; the fp16 adds trail
                mh = {}
                for half in (1, 0):
                    k0 = 2 * half
                    if s > 0:
                        m_h = ew2.tile([128, 2, BPC], FP16, tag=f"m{half}",
                                       name=f"m{half}")
                        nc.vector.tensor_tensor(
                            m_h[:], a_sb[:, k0:k0 + 2], n_sb[:, k0:k0 + 2],
                            OP.mult)
                        mh[half] = m_h
                        if not last:
                            nc.vector.tensor_tensor(
                                nhh[:, k0:k0 + 2], p_sb[:, k0:k0 + 2],
                                m_h[:], OP.add)
                    elif not last:
                        nc.vector.tensor_tensor(
                            nhh[:, k0:k0 + 2], a_sb[:, k0:k0 + 2],
                            n_sb[:, k0:k0 + 2], OP.mult)
                for half in (1, 0):
                    k0 = 2 * half
                    if s == 0:
                        nc.vector.tensor_tensor(
                            nh[:, k0:k0 + 2], a_sb[:, k0:k0 + 2],
                            n_sb[:, k0:k0 + 2], OP.mult)
                    else:
                        nc.vector.tensor_tensor(
                            nh[:, k0:k0 + 2], p_sb[:, k0:k0 + 2],
                            mh[half][:], OP.add)
                h16 = nh
                h8h_t = nhh

                # ---- x-part matmuls for s+1 ----
                if s + 1 < S:
                    pr = ps_r.tile([128, 4, BPC], FP32, tag="pr", name="pr")
                    pz = ps_z.tile([128, 4, BPC], FP32, tag="pz", name="pz")
                    pxn = ps_x.tile([128, 4, BPC], FP32, tag="pxn",
                                    name="pxn")
                    x_mms(cbuf[s + 1], [(pr, 0), (pxn, 2), (pz, 1)],
                          with_stop=False)
                    # stage the n-gate x psum to fp16 SBUF off the critical
                    # path (tn2 then runs in DVE fast mode)
                    x16n = xnp.tile([128, 4, BPC], FP16, tag="x16n",
                                    name="x16n")
                    nc.scalar.activation(x16n[:], pxn[:], AF.Copy)
                    del cbuf[s + 1]
                    if s + 5 < S:
                        cbuf[s + 5] = load_c(s + 5)

                # ---- interleaved scoring ----
                # group 1 (needed from s=4) is scored during steps 0-3
                if s < 4:
                    fc1_chunk(1, s)
                gisc = s // SGRP + 2
                q = s % SGRP
                if gisc <= NGRP - 1:
                    if q == 0 and gisc + 1 <= NGRP - 1:
                        load_feat(gisc + 1)
                    fc1_chunk(gisc, q)

            # ================= epilogue =================
            # transpose + copy + store per kt-half as the last chain's
            # halves land
            pt = ps_r.tile([128, 4, BPC], FP32, tag="pr", name="ptr")
            ot = ew.tile([128, H], FP32, tag="ot")
            for half in (1, 0):
                for kt in (2 * half, 2 * half + 1):
                    nc.tensor.transpose(pt[:, kt], h16[:, kt], idt[:])
                nc.scalar.activation(
                    ot[:, 256 * half:256 * half + 256],
                    pt[:, 2 * half:2 * half + 2].rearrange(
                        "p k b -> p (k b)"), AF.Copy)
                nc.sync.dma_start(
                    out.ap()[:, 256 * half:256 * half + 256],
                    ot[:, 256 * half:256 * half + 256])

    nc.compile()
    return nc


def _prep(C, Q, prev_M, fc1_w, fc2_w, W_ih, W_hh):
    """Host-side layout/dtype transforms + per-core sharding."""
    consts = {}
    consts["ident"] = np.eye(128, dtype=np.float32)
    # W_ih^T hi/lo fp8 in DR lhsT layout [p, pair, gate-tile, i, 128]
    hi, lo = _split8(np.ascontiguousarray(W_ih.T), 16.0)  # [K, G3]
    for nm, t in (("wih8h", hi), ("wih8l", lo)):
        consts[nm] = np.ascontiguousarray(
            t.reshape(2, 2, 128, NT, 128).transpose(2, 0, 3, 1, 4))
    # W_hh^T * 512 hi/lo fp8, same layout (h quantized at scale 1.0)
    hi, lo = _split8(np.ascontiguousarray(W_hh.T), 512.0)  # [H, G3]
    for nm, t in (("whh8h", hi), ("whh8l", lo)):
        consts[nm] = np.ascontiguousarray(
            t.reshape(2, 2, 128, NT, 128).transpose(2, 0, 3, 1, 4))
    f1p = np.zeros((128, 4 * H), np.float32)  # pad SH 120 -> 128
    f1p[:SH] = fc1_w
    hi, lo = _split8(np.ascontiguousarray(f1p.T), 16.0)  # [K, 128]
    for nm, t in (("f18h", hi), ("f18l", lo)):
        consts[nm] = np.ascontiguousarray(
            t.reshape(FK // 2, 2, 128, 128).transpose(2, 0, 1, 3))
    consts["f2t16"] = np.ascontiguousarray(fc2_w.T).astype(F16)

    in_maps = []
    for c in range(NCORES):
        lo_, hi_ = c * BPC, (c + 1) * BPC
        Cc = np.ascontiguousarray(C[lo_:hi_])          # [BPC, S, H]
        Qc = Q[lo_:hi_, 0]                              # [BPC, H]
        Mc = prev_M[lo_:hi_, 0]
        m = {}
        Ct = np.ascontiguousarray(Cc.transpose(1, 2, 0))   # [S, H, BPC]
        chi, clo = _split8(Ct, 32.0)  # [S, H, BPC]
        both = np.stack([chi, clo])   # [2, S, H, BPC]
        m["c8"] = np.ascontiguousarray(
            both.reshape(2, S, 2, 2, 128, BPC).transpose(0, 1, 4, 2, 3, 5))
        # feat [BPC, S, 4H]
        feat = np.concatenate(
            [Cc * Qc[:, None, :], Cc * Mc[:, None, :],
             np.abs(Cc - Qc[:, None, :]), np.abs(Cc - Mc[:, None, :])],
            axis=2)
        # -> [NGRP, 4H, SGRP, BPC]
        ftr = np.ascontiguousarray(
            feat.transpose(1, 2, 0).reshape(NGRP, SGRP, 4 * H, BPC)
            .transpose(0, 2, 1, 3))
        fhi, flo = _split8(ftr, 8.0)       # [NGRP, 4H, SGRP, BPC]
        both = np.stack([fhi, flo])
        m["feat8"] = np.ascontiguousarray(
            both.reshape(2, NGRP, FK // 2, 2, 128, SGRP * BPC)
            .transpose(0, 1, 4, 2, 3, 5))
        in_maps.append(m)
    return consts, in_maps


def kernel(C, Q, prev_M, fc1_w, fc1_b, fc2_w, fc2_b, W_ih, W_hh, b_ih, b_hh):
    from concourse.bass_utils import run_bass_kernel_spmd

    C = np.asarray(C, dtype=np.float32)
    Q = np.asarray(Q, dtype=np.float32)
    prev_M = np.asarray(prev_M, dtype=np.float32)
    fc1_w = np.asarray(fc1_w, np.float32)
    fc2_w = np.asarray(fc2_w, np.float32)
    W_ih = np.asarray(W_ih, np.float32)
    W_hh = np.asarray(W_hh, np.float32)
    fc1_b = np.asarray(fc1_b, np.float32)
    fc2_b = np.asarray(fc2_b, np.float32)
    b_ih = np.asarray(b_ih, np.float32)
    b_hh = np.asarray(b_hh, np.float32)
    assert not (np.any(fc1_b) or np.any(fc2_b) or np.any(b_ih)
                or np.any(b_hh)), "nonzero biases unsupported in v3"

    consts, in_maps = _prep(C, Q, prev_M, fc1_w, fc2_w, W_ih, W_hh)

    key = tuple(np.asarray(v).tobytes() for v in consts.values())
    kh = hash(key)
    if kh not in _CACHE:
        _CACHE[kh] = _build(consts)
    nc = _CACHE[kh]

    res = run_bass_kernel_spmd(nc, in_maps, list(range(NCORES)))
    h = np.concatenate([res.results[c]["out"] for c in range(NCORES)],
                       axis=0)
    return h[:, None, :].astype(np.float32)


# revision 40
# speedup vs baseline: 1.2066x; 1.0011x over previous
"""EpisodicMemory Trainium2 kernel, v3.

Data-parallel over batch across 8 NeuronCores (128 batch rows per core).

Layout is "flipped": the GRU state h and all gate pre-activations live as
[H-on-partitions (4 k-tiles of 128), batch-on-free(128)] tiles, so the
recurrent matmul h @ W_hh^T needs NO transposes: its rhs (moving operand)
is h itself, and the elementwise update produces h directly in that
layout. Per-sentence episodic gates are broadcast across partitions with a
stride-0 DMA from DRAM.

Precision plan (validated offline, rel err ~1.3e-2 vs 2e-2 budget):
 - x-part (C @ W_ih^T) and scoring fc1: 3-term error-compensated fp8-e4m3
   DoubleRow matmuls (hi/lo splits of both operands, Whi@Chi + Whi@Clo +
   Wlo@Chi), 0.5 cyc/row with K=256 per instruction.
 - h-part (h @ W_hh^T): fp8 DoubleRow, every gate 2-term WEIGHT-side
   compensated ((Whi+Wlo) @ hhi) so only the hi fp8 plane of h is ever
   needed. h stays in fp16 (quantized at scale 1.0, |h|<=1); the chain's
   p+m add is issued twice, fp8-out first (h8h) so the next step's h-part
   matmuls unblock before the fp16 adds complete.
 - tn/tn2 (r*hn + xn) are fp16 at the 512x psum scale; the n-gate x psum
   is pre-staged to fp16 SBUF (ACT copy, off the critical path) so tn2
   runs in the DVE 2x mode instead of reading psum at 1x.
 - All W matrices pre-scaled so every GRU psum holds 512x the preact
   (fc1: 128x); the 1/512 folds into the ACT sigmoid/tanh scale.
 - C / feat = [C*Q, C*prev_M, |C-Q|, |C-prev_M|] quantized host-side.

Per-step critical loop (the scan is latency-bound, not engine-bound):
h8h -> r/n/z h-matmuls -> sigmoid(r) -> tn -> tn2 -> tanh (2 chunks) ->
m -> h8h. DVE work is kept just under the loop period; a/t1/p fill the
tanh window. Startup: tiny consts DMA first, then fc1 weights + feat
group 0 in interleaved hi/lo quarters so scoring matmuls start ~4us in;
identity-tile warmup matmuls ramp the PE p-state during the DMA wait.
"""
import numpy as np
import ml_dtypes

H = 512
SH = 120
B = 1024
S = 64
NCORES = 8
BPC = B // NCORES  # 128
KH = H // 128      # 4
G3 = 3 * H
SGRP = 4
NGRP = S // SGRP   # 16
NT = G3 // 128     # 12 gate tiles
FK = 4 * H // 128  # 16 feat k-tiles
E4 = ml_dtypes.float8_e4m3
F16 = np.float16

N_WARMUP = 10      # identity matmuls to ramp the PE p-state during DMA wait

_CACHE = {}


def _q8(x):
    return np.clip(np.asarray(x, np.float32), -240.0, 240.0).astype(E4)


def _split8(x, scale):
    hi = _q8(x * scale)
    lo = _q8(x * scale - hi.astype(np.float32))
    return hi, lo


def _build(consts):
    import concourse.bass as bass
    import concourse.tile as tile
    from concourse import bacc, mybir

    FP32 = mybir.dt.float32
    FP16 = mybir.dt.float16
    FP8 = mybir.dt.float8e4
    OP = mybir.AluOpType
    AF = mybir.ActivationFunctionType
    PM = mybir.MatmulPerfMode

    nc = bacc.Bacc("TRN2", target_bir_lowering=False, debug=False,
                   num_devices=NCORES)

    # ---- external inputs (per core) ----
    c_t = nc.dram_tensor("c8", [2, S, 128, 2, 2, BPC], FP8,
                         kind="ExternalInput")  # [hi/lo, s, p, pair, i, b]
    f_t = nc.dram_tensor("feat8", [2, NGRP, 128, FK // 2, 2, SGRP * BPC],
                         FP8, kind="ExternalInput")
    out = nc.dram_tensor("out", [BPC, H], FP32, kind="ExternalOutput")

    # ---- inline consts ----
    dl = {}
    for k, v in consts.items():
        dl[k] = nc.inline_tensor(v, name=k)

    from contextlib import ExitStack
    with tile.TileContext(nc) as tc:
        with ExitStack() as ctx:
            cpool = ctx.enter_context(tc.tile_pool(name="const", bufs=1))
            cpool2 = ctx.enter_context(tc.tile_pool(name="const2", bufs=1))
            cstr = ctx.enter_context(tc.tile_pool(name="cstr", bufs=6))
            fstr = ctx.enter_context(tc.tile_pool(name="fstr", bufs=3))
            hpool = ctx.enter_context(tc.tile_pool(name="h", bufs=3))
            hq = ctx.enter_context(tc.tile_pool(name="hq", bufs=4))
            gpool = ctx.enter_context(tc.tile_pool(name="g", bufs=4))
            gdram = ctx.enter_context(tc.tile_pool(name="gd", bufs=4,
                                                   space="DRAM"))
            ew = ctx.enter_context(tc.tile_pool(name="ew", bufs=3))
            ew2 = ctx.enter_context(tc.tile_pool(name="ew2", bufs=3))
            xnp = ctx.enter_context(tc.tile_pool(name="xn16", bufs=3))
            sco = ctx.enter_context(tc.tile_pool(name="sco", bufs=2))
            ps_r = ctx.enter_context(tc.tile_pool(name="ps_r", bufs=2,
                                                  space="PSUM"))
            ps_z = ctx.enter_context(tc.tile_pool(name="ps_z", bufs=2,
                                                  space="PSUM"))
            ps_x = ctx.enter_context(tc.tile_pool(name="ps_x", bufs=1,
                                                  space="PSUM"))
            ps_h = ctx.enter_context(tc.tile_pool(name="ps_h", bufs=1,
                                                  space="PSUM"))
            ps_f = ctx.enter_context(tc.tile_pool(name="ps_f", bufs=2,
                                                  space="PSUM"))

            # ---- tiny consts first: identity (warmup + epilogue), fc2 ----
            idt = cpool.tile([128, 128], FP32, tag="idt")
            nc.sync.dma_start(idt[:], dl["ident"].ap())
            f2t = cpool2.tile([SH, 1], FP16, tag="f2t")
            nc.sync.dma_start(f2t[:], dl["f2t16"].ap())
            # touch every activation function once so the ACT table loads
            # happen during the const-DMA wait, not on the scan chain
            warm = cpool2.tile([1, 4], FP32, tag="warm")
            for af in (AF.Sigmoid, AF.Tanh, AF.Copy):
                nc.scalar.activation(warm[:], idt[0:1, 0:4], af)
            # PE p-state warmup: fp32 identity matmuls (512 cyc each), no
            # data deps beyond idt, so they run while the big DMAs stream.
            # Aliases the fc1 psum tag: PE is in-order, so no extra blocking.
            wps = ps_f.tile([128, SGRP * BPC], FP32, tag="pps", name="wps")
            for i in range(N_WARMUP):
                nc.tensor.matmul(wps[:, 0:128], idt[:], idt[:],
                                 start=(i == 0), stop=(i == N_WARMUP - 1))

            # ---- fc1 weights, then feat g0 (quartered) so scoring starts
            # as early as possible ----
            f1h = cpool2.tile([128, FK // 2, 2, 128], FP8, tag="f1h")
            nc.sync.dma_start(f1h[:], dl["f18h"].ap())
            f1l = cpool2.tile([128, FK // 2, 2, 128], FP8, tag="f1l")
            nc.sync.dma_start(f1l[:], dl["f18l"].ap())

            # ================= helpers =================
            def load_c(s):
                ch = cstr.tile([128, 2, 2, BPC], FP8, tag="csh")
                nc.sync.dma_start(ch[:], c_t.ap()[0, s])
                cl = cstr.tile([128, 2, 2, BPC], FP8, tag="csl")
                nc.sync.dma_start(cl[:], c_t.ap()[1, s])
                return (ch, cl)

            def x_mms(cts, banks, with_stop):
                """x-part matmuls for one sentence into the given
                [(psum, gate)] banks. pr/pz use per-jj-half accumulation
                regions (the chain's sigmoid chunks read each half as soon
                as the h-part closes it); pxn is a single region, closed
                here (the h-part never accumulates into it)."""
                ch, cl = cts
                for pb, gate in banks:
                    for jj in range(4):
                        gt_ = gate * 4 + jj
                        for pair in range(2):
                            terms = [(wih_h, ch), (wih_h, cl),
                                     (wih_l, ch)]
                            for ti, (wt_, ct_) in enumerate(terms):
                                nc.tensor.matmul(
                                    pb[:, jj],
                                    wt_[:, pair, gt_],
                                    ct_[:, pair],
                                    start=(jj == 0 and pair == 0
                                           and ti == 0),
                                    stop=((with_stop or gate == 2)
                                          and jj == 3
                                          and pair == 1 and ti == 2),
                                    perf_mode=PM.DoubleRow)

            def h_mms(h8h_t, pr, pz, pxn, phn):
                """h-part fp8 DR matmuls: every gate is 2-term weight-side
                compensated ((Whi+Wlo) @ hhi), so only the hi plane of h is
                needed — the chain never produces a lo plane and the n-gate
                psum closes right after the hi plane lands. Gate order
                r, n, z: r feeds the sigmoid, n closes phn for tn, z last."""
                for gate, pb in ((0, pr), (2, phn), (1, pz)):
                    # r gate: 1 term (Whi @ hhi) — halving its matmul block
                    # lets sigmoid(r), which is on the serial loop, start
                    # ~200ns earlier. z/n: 2-term weight-side.
                    terms = [whh_h] if gate == 0 else [whh_h, whh_l]
                    for ti, wt_ in enumerate(terms):
                        for pair in (1, 0):
                            for jj in range(4):
                                nc.tensor.matmul(
                                    pb[:, jj], wt_[:, pair, 4 * gate + jj],
                                    h8h_t[:, 2 * pair:2 * pair + 2],
                                    start=(gate == 2 and ti == 0
                                           and pair == 1 and jj == 0),
                                    stop=(ti == len(terms) - 1
                                          and pair == 0 and jj == 3),
                                    perf_mode=PM.DoubleRow)

            # ---- scoring machinery ----
            grp = {}

            def load_feat(gi, quarters=False):
                # split loads along k so downstream fc1 chunks unblock
                # progressively; interleave hi/lo quarters for the startup
                # groups so chunk q unblocks after 1/4 of the bytes
                fh = fstr.tile([128, FK // 2, 2, SGRP * BPC], FP8,
                               tag="feath")
                fl = fstr.tile([128, FK // 2, 2, SGRP * BPC], FP8,
                               tag="featl")
                if quarters:
                    for q0 in range(0, 8, 2):
                        nc.sync.dma_start(fh[:, q0:q0 + 2],
                                          f_t.ap()[0, gi, :, q0:q0 + 2])
                        nc.sync.dma_start(fl[:, q0:q0 + 2],
                                          f_t.ap()[1, gi, :, q0:q0 + 2])
                else:
                    nc.sync.dma_start(fh[:, 0:4], f_t.ap()[0, gi, :, 0:4])
                    nc.sync.dma_start(fh[:, 4:8], f_t.ap()[0, gi, :, 4:8])
                    nc.sync.dma_start(fl[:, 0:4], f_t.ap()[1, gi, :, 0:4])
                    nc.sync.dma_start(fl[:, 4:8], f_t.ap()[1, gi, :, 4:8])
                grp[gi] = {"feat": (fh, fl)}

            def fc1_chunk(gi, q):
                """Quarter q of group gi's fc1 matmuls."""
                st = grp[gi]
                if q == 0:
                    st["pps"] = ps_f.tile([128, SGRP * BPC], FP32, tag="pps",
                                          name="pps")
                pps = st["pps"]
                fh, fl = st["feat"]
                for pair in range(2 * q, 2 * q + 2):
                    terms = [(f1h, fh), (f1h, fl), (f1l, fh)]
                    for ti, (wt_, ft_) in enumerate(terms):
                        nc.tensor.matmul(
                            pps[:], wt_[:, pair], ft_[:, pair],
                            start=(pair == 0 and ti == 0),
                            stop=(pair == FK // 2 - 1 and ti == 2),
                            perf_mode=PM.DoubleRow)
                if q == 3:
                    finish_group(gi)

            def finish_group(gi):
                st = grp[gi]
                pps = st["pps"]
                h1 = sco.tile([SH, SGRP * BPC], FP16, tag="h1")
                nc.scalar.activation(h1[:], pps[0:SH, :], AF.Tanh,
                                     scale=1.0 / 128)
                nc.tensor.matmul(pps[0:1, :], f2t[:], h1[:],
                                 start=True, stop=True)
                gt = gpool.tile([1, SGRP * BPC], FP16, tag="gt")
                nc.scalar.activation(gt[:], pps[0:1, :], AF.Sigmoid)
                gd = gdram.tile([1, SGRP * BPC], FP16, tag="gd")
                nc.sync.dma_start(gd[:], gt[:])
                grep = gpool.tile([128, SGRP, BPC], FP16, tag="grep")
                nc.sync.dma_start(
                    grep[:], gd[:].broadcast_to([128, SGRP * BPC]))
                st["grep"] = grep
                del st["pps"], st["feat"]

            # ================= prologue =================
            load_feat(0, quarters=True)
            for q in range(4):
                fc1_chunk(0, q)

            wih_h = cpool.tile([128, 2, NT, 2, 128], FP8, tag="wih_h")
            nc.sync.dma_start(wih_h[:], dl["wih8h"].ap())
            wih_l = cpool.tile([128, 2, NT, 2, 128], FP8, tag="wih_l")
            nc.sync.dma_start(wih_l[:], dl["wih8l"].ap())
            cbuf = {0: load_c(0)}
            pz = ps_z.tile([128, 4, BPC], FP32, tag="pz", name="pz")
            pxn = ps_x.tile([128, 4, BPC], FP32, tag="pxn", name="pxn")
            x_mms(cbuf[0], [(pxn, 2), (pz, 1)], with_stop=True)
            del cbuf[0]

            # recurrent weights right behind c0 (step 1's h-part gates on
            # whh_h); c1/c2 slot between the two planes so early x-matmuls
            # are not starved; group 1's scoring runs inside scan steps 0-3
            whh_h = cpool.tile([128, 2, NT, 2, 128], FP8, tag="whh_h")
            nc.sync.dma_start(whh_h[:], dl["whh8h"].ap())
            cbuf[1] = load_c(1)
            whh_l = cpool.tile([128, 2, NT, 2, 128], FP8, tag="whh_l")
            nc.sync.dma_start(whh_l[:], dl["whh8l"].ap())
            for s in range(2, 5):
                cbuf[s] = load_c(s)
            load_feat(1, quarters=True)
            load_feat(2)

            h16 = None
            h8h_t = None
            pr = None
            ISC = 1.0 / 512

            # ================= scan =================
            for s in range(S):
                # ---- h-part matmuls (s>0) ----
                if s > 0:
                    phn = ps_h.tile([128, 4, BPC], FP32, tag="phn",
                                    name="phn")
                    h_mms(h8h_t, pr, pz, pxn, phn)

                # ---- elementwise chain ----
                gi = s // SGRP
                j = s % SGRP
                grep = grp[gi]["grep"]

                if s > 0:
                    r_sb = ew.tile([128, 4, BPC], FP16, tag="r")
                    nc.scalar.activation(r_sb[:], pr[:], AF.Sigmoid,
                                         scale=ISC)
                    # tn/tn2 in fp16 at the 512x psum scale: tn2's second
                    # operand is the pre-copied fp16 xn, so the add runs in
                    # DVE fast mode instead of reading psum at 1x
                    tn = ew2.tile([128, 4, BPC], FP16, tag="tn")
                    nc.vector.tensor_tensor(tn[:], r_sb[:], phn[:], OP.mult)
                w_sb = ew.tile([128, 4, BPC], FP16, tag="w")
                nc.scalar.activation(w_sb[:], pz[:], AF.Sigmoid, scale=-ISC)
                if s > 0:
                    tn2 = ew2.tile([128, 4, BPC], FP16, tag="tn2")
                    nc.vector.tensor_tensor(tn2[:], tn[:], x16n[:], OP.add)
                a_sb = ew.tile([128, 4, BPC], FP16, tag="a")
                nc.vector.tensor_tensor(
                    a_sb[:], w_sb[:],
                    grep[:, j].unsqueeze(1).broadcast_to([128, 4, BPC]),
                    OP.mult)
                # a / t1 / p are off the critical chain (they fill the DVE
                # while tanh runs on ACT)
                if s > 0:
                    t1 = ew2.tile([128, 4, BPC], FP16, tag="t1")
                    nc.vector.tensor_tensor(t1[:], a_sb[:], h16[:], OP.mult)
                    p_sb = ew2.tile([128, 4, BPC], FP16, tag="p")
                    nc.vector.tensor_tensor(p_sb[:], h16[:], t1[:],
                                            OP.subtract)
                n_sb = ew.tile([128, 4, BPC], FP16, tag="n")
                CHT = ((2, 4), (0, 2))
                for c0_, c1_ in CHT:
                    if s > 0:
                        nc.scalar.activation(n_sb[:, c0_:c1_],
                                             tn2[:, c0_:c1_], AF.Tanh,
                                             scale=ISC)
                    else:
                        nc.scalar.activation(n_sb[:, c0_:c1_],
                                             pxn[:, c0_:c1_], AF.Tanh,
                                             scale=ISC)
                last = s == S - 1
                if last:
                    nh = ew.tile([128, KH, BPC], FP32, tag="hf")
                    nhh = None
                else:
                    nh = hpool.tile([128, KH, BPC], FP16, tag="h", name="h")
                    nhh = hq.tile([128, KH, BPC], FP8, tag="h8h",
                                  name="h8h")
                # post-tanh tail, all on DVE (no cross-engine hops): the fp8
                # hi-plane adds come FIRST — they alone unblock ALL of next
                # step's h-part matmuls; the fp16 adds trail
                mh = {}
                for half in (1, 0):
                    k0 = 2 * half
                    if s > 0:
                        m_h = ew2.tile([128, 2, BPC], FP16, tag=f"m{half}",
                                       name=f"m{half}")
                        nc.vector.tensor_tensor(
                            m_h[:], a_sb[:, k0:k0 + 2], n_sb[:, k0:k0 + 2],
                            OP.mult)
                        mh[half] = m_h
                        if not last:
                            nc.vector.tensor_tensor(
                                nhh[:, k0:k0 + 2], p_sb[:, k0:k0 + 2],
                                m_h[:], OP.add)
                    elif not last:
                        nc.vector.tensor_tensor(
                            nhh[:, k0:k0 + 2], a_sb[:, k0:k0 + 2],
                            n_sb[:, k0:k0 + 2], OP.mult)
                for half in (1, 0):
                    k0 = 2 * half
                    if s == 0:
                        nc.vector.tensor_tensor(
                            nh[:, k0:k0 + 2], a_sb[:, k0:k0 + 2],
                            n_sb[:, k0:k0 + 2], OP.mult)
                    else:
                        nc.vector.tensor_tensor(
                            nh[:, k0:k0 + 2], p_sb[:, k0:k0 + 2],
                            mh[half][:], OP.add)
                h16 = nh
                h8h_t = nhh

                # ---- x-part matmuls for s+1 ----
                if s + 1 < S:
                    pr = ps_r.tile([128, 4, BPC], FP32, tag="pr", name="pr")
                    pz = ps_z.tile([128, 4, BPC], FP32, tag="pz", name="pz")
                    pxn = ps_x.tile([128, 4, BPC], FP32, tag="pxn",
                                    name="pxn")
                    x_mms(cbuf[s + 1], [(pr, 0), (pxn, 2), (pz, 1)],
                          with_stop=False)
                    # stage the n-gate x psum to fp16 SBUF off the critical
                    # path (tn2 then runs in DVE fast mode)
                    x16n = xnp.tile([128, 4, BPC], FP16, tag="x16n",
                                    name="x16n")
                    nc.scalar.activation(x16n[:], pxn[:], AF.Copy)
                    del cbuf[s + 1]
                    if s + 5 < S:
                        cbuf[s + 5] = load_c(s + 5)

                # ---- interleaved scoring ----
                # group 1 (needed from s=4) is scored during steps 0-3
                if s < 4:
                    fc1_chunk(1, s)
                gisc = s // SGRP + 2
                q = s % SGRP
                if gisc <= NGRP - 1:
                    if q == 0 and gisc + 1 <= NGRP - 1:
                        load_feat(gisc + 1)
                    fc1_chunk(gisc, q)

            # ================= epilogue =================
            # transpose + copy + store per kt-half as the last chain's
            # halves land
            pt = ps_r.tile([128, 4, BPC], FP32, tag="pr", name="ptr")
            ot = ew.tile([128, H], FP32, tag="ot")
            for half in (1, 0):
                for kt in (2 * half, 2 * half + 1):
                    nc.tensor.transpose(pt[:, kt], h16[:, kt], idt[:])
                nc.scalar.activation(
                    ot[:, 256 * half:256 * half + 256],
                    pt[:, 2 * half:2 * half + 2].rearrange(
                        "p k b -> p (k b)"), AF.Copy)
                nc.sync.dma_start(
                    out.ap()[:, 256 * half:256 * half + 256],
                    ot[:, 256 * half:256 * half + 256])

    nc.compile()
    return nc


def _prep(C, Q, prev_M, fc1_w, fc2_w, W_ih, W_hh):
    """Host-side layout/dtype transforms + per-core sharding."""
    consts = {}
    consts["ident"] = np.eye(128, dtype=np.float32)
    # W_ih^T hi/lo fp8 in DR lhsT layout [p, pair, gate-tile, i, 128]
    hi, lo = _split8(np.ascontiguousarray(W_ih.T), 16.0)  # [K, G3]
    for nm, t in (("wih8h", hi), ("wih8l", lo)):
        consts[nm] = np.ascontiguousarray(
            t.reshape(2, 2, 128, NT, 128).transpose(2, 0, 3, 1, 4))
    # W_hh^T * 512 hi/lo fp8, same layout (h quantized at scale 1.0)
    hi, lo = _split8(np.ascontiguousarray(W_hh.T), 512.0)  # [H, G3]
    for nm, t in (("whh8h", hi), ("whh8l", lo)):
        consts[nm] = np.ascontiguousarray(
            t.reshape(2, 2, 128, NT, 128).transpose(2, 0, 3, 1, 4))
    f1p = np.zeros((128, 4 * H), np.float32)  # pad SH 120 -> 128
    f1p[:SH] = fc1_w
    hi, lo = _split8(np.ascontiguousarray(f1p.T), 16.0)  # [K, 128]
    for nm, t in (("f18h", hi), ("f18l", lo)):
        consts[nm] = np.ascontiguousarray(
            t.reshape(FK // 2, 2, 128, 128).transpose(2, 0, 1, 3))
    consts["f2t16"] = np.ascontiguousarray(fc2_w.T).astype(F16)

    in_maps = []
    for c in range(NCORES):
        lo_, hi_ = c * BPC, (c + 1) * BPC
        Cc = np.ascontiguousarray(C[lo_:hi_])          # [BPC, S, H]
        Qc = Q[lo_:hi_, 0]                              # [BPC, H]
        Mc = prev_M[lo_:hi_, 0]
        m = {}
        Ct = np.ascontiguousarray(Cc.transpose(1, 2, 0))   # [S, H, BPC]
        chi, clo = _split8(Ct, 32.0)  # [S, H, BPC]
        both = np.stack([chi, clo])   # [2, S, H, BPC]
        m["c8"] = np.ascontiguousarray(
            both.reshape(2, S, 2, 2, 128, BPC).transpose(0, 1, 4, 2, 3, 5))
        # feat [BPC, S, 4H]
        feat = np.concatenate(
            [Cc * Qc[:, None, :], Cc * Mc[:, None, :],
             np.abs(Cc - Qc[:, None, :]), np.abs(Cc - Mc[:, None, :])],
            axis=2)
        # -> [NGRP, 4H, SGRP, BPC]
        ftr = np.ascontiguousarray(
            feat.transpose(1, 2, 0).reshape(NGRP, SGRP, 4 * H, BPC)
            .transpose(0, 2, 1, 3))
        fhi, flo = _split8(ftr, 8.0)       # [NGRP, 4H, SGRP, BPC]
        both = np.stack([fhi, flo])
        m["feat8"] = np.ascontiguousarray(
            both.reshape(2, NGRP, FK // 2, 2, 128, SGRP * BPC)
            .transpose(0, 1, 4, 2, 3, 5))
        in_maps.append(m)
    return consts, in_maps


def kernel(C, Q, prev_M, fc1_w, fc1_b, fc2_w, fc2_b, W_ih, W_hh, b_ih, b_hh):
    from concourse.bass_utils import run_bass_kernel_spmd

    C = np.asarray(C, dtype=np.float32)
    Q = np.asarray(Q, dtype=np.float32)
    prev_M = np.asarray(prev_M, dtype=np.float32)
    fc1_w = np.asarray(fc1_w, np.float32)
    fc2_w = np.asarray(fc2_w, np.float32)
    W_ih = np.asarray(W_ih, np.float32)
    W_hh = np.asarray(W_hh, np.float32)
    fc1_b = np.asarray(fc1_b, np.float32)
    fc2_b = np.asarray(fc2_b, np.float32)
    b_ih = np.asarray(b_ih, np.float32)
    b_hh = np.asarray(b_hh, np.float32)
    assert not (np.any(fc1_b) or np.any(fc2_b) or np.any(b_ih)
                or np.any(b_hh)), "nonzero biases unsupported in v3"

    consts, in_maps = _prep(C, Q, prev_M, fc1_w, fc2_w, W_ih, W_hh)

    key = tuple(np.asarray(v).tobytes() for v in consts.values())
    kh = hash(key)
    if kh not in _CACHE:
        _CACHE[kh] = _build(consts)
    nc = _CACHE[kh]

    res = run_bass_kernel_spmd(nc, in_maps, list(range(NCORES)))
    h = np.concatenate([res.results[c]["out"] for c in range(NCORES)],
                       axis=0)
    return h[:, None, :].astype(np.float32)


# revision 44
# speedup vs baseline: 1.2069x; 1.0003x over previous
"""EpisodicMemory Trainium2 kernel, v3.

Data-parallel over batch across 8 NeuronCores (128 batch rows per core).

Layout is "flipped": the GRU state h and all gate pre-activations live as
[H-on-partitions (4 k-tiles of 128), batch-on-free(128)] tiles, so the
recurrent matmul h @ W_hh^T needs NO transposes: its rhs (moving operand)
is h itself, and the elementwise update produces h directly in that
layout. Per-sentence episodic gates are broadcast across partitions with a
stride-0 DMA from DRAM.

Precision plan (validated offline, rel err ~1.3e-2 vs 2e-2 budget):
 - x-part (C @ W_ih^T) and scoring fc1: 3-term error-compensated fp8-e4m3
   DoubleRow matmuls (hi/lo splits of both operands, Whi@Chi + Whi@Clo +
   Wlo@Chi), 0.5 cyc/row with K=256 per instruction.
 - h-part (h @ W_hh^T): fp8 DoubleRow, every gate 2-term WEIGHT-side
   compensated ((Whi+Wlo) @ hhi) so only the hi fp8 plane of h is ever
   needed. h stays in fp16 (quantized at scale 1.0, |h|<=1); the chain's
   p+m add is issued twice, fp8-out first (h8h) so the next step's h-part
   matmuls unblock before the fp16 adds complete.
 - tn/tn2 (r*hn + xn) are fp16 at the 512x psum scale; the n-gate x psum
   is pre-staged to fp16 SBUF (ACT copy, off the critical path) so tn2
   runs in the DVE 2x mode instead of reading psum at 1x.
 - All W matrices pre-scaled so every GRU psum holds 512x the preact
   (fc1: 128x); the 1/512 folds into the ACT sigmoid/tanh scale.
 - C / feat = [C*Q, C*prev_M, |C-Q|, |C-prev_M|] quantized host-side.

Per-step critical loop (the scan is latency-bound, not engine-bound):
h8h -> r/n/z h-matmuls -> sigmoid(r) -> tn -> tn2 -> tanh (2 chunks) ->
m -> h8h. DVE work is kept just under the loop period; a/t1/p fill the
tanh window. Startup: tiny consts DMA first, then fc1 weights + feat
group 0 in interleaved hi/lo quarters so scoring matmuls start ~4us in;
identity-tile warmup matmuls ramp the PE p-state during the DMA wait.
"""
import numpy as np
import ml_dtypes

H = 512
SH = 120
B = 1024
S = 64
NCORES = 8
BPC = B // NCORES  # 128
KH = H // 128      # 4
G3 = 3 * H
SGRP = 4
NGRP = S // SGRP   # 16
NT = G3 // 128     # 12 gate tiles
FK = 4 * H // 128  # 16 feat k-tiles
E4 = ml_dtypes.float8_e4m3
F16 = np.float16

N_WARMUP = 10      # identity matmuls to ramp the PE p-state during DMA wait

_CACHE = {}


def _q8(x):
    return np.clip(np.asarray(x, np.float32), -240.0, 240.0).astype(E4)


def _split8(x, scale):
    hi = _q8(x * scale)
    lo = _q8(x * scale - hi.astype(np.float32))
    return hi, lo


def _build(consts):
    import concourse.bass as bass
    import concourse.tile as tile
    from concourse import bacc, mybir

    FP32 = mybir.dt.float32
    FP16 = mybir.dt.float16
    FP8 = mybir.dt.float8e4
    OP = mybir.AluOpType
    AF = mybir.ActivationFunctionType
    PM = mybir.MatmulPerfMode

    nc = bacc.Bacc("TRN2", target_bir_lowering=False, debug=False,
                   num_devices=NCORES)

    # ---- external inputs (per core) ----
    c_t = nc.dram_tensor("c8", [2, S, 128, 2, 2, BPC], FP8,
                         kind="ExternalInput")  # [hi/lo, s, p, pair, i, b]
    f_t = nc.dram_tensor("feat8", [2, NGRP, 128, FK // 2, 2, SGRP * BPC],
                         FP8, kind="ExternalInput")
    out = nc.dram_tensor("out", [BPC, H], FP32, kind="ExternalOutput")

    # ---- inline consts ----
    dl = {}
    for k, v in consts.items():
        dl[k] = nc.inline_tensor(v, name=k)

    from contextlib import ExitStack
    with tile.TileContext(nc) as tc:
        with ExitStack() as ctx:
            cpool = ctx.enter_context(tc.tile_pool(name="const", bufs=1))
            cpool2 = ctx.enter_context(tc.tile_pool(name="const2", bufs=1))
            cstr = ctx.enter_context(tc.tile_pool(name="cstr", bufs=6))
            fstr = ctx.enter_context(tc.tile_pool(name="fstr", bufs=3))
            hpool = ctx.enter_context(tc.tile_pool(name="h", bufs=3))
            hq = ctx.enter_context(tc.tile_pool(name="hq", bufs=4))
            gpool = ctx.enter_context(tc.tile_pool(name="g", bufs=4))
            gdram = ctx.enter_context(tc.tile_pool(name="gd", bufs=4,
                                                   space="DRAM"))
            ew = ctx.enter_context(tc.tile_pool(name="ew", bufs=3))
            ew2 = ctx.enter_context(tc.tile_pool(name="ew2", bufs=3))
            xnp = ctx.enter_context(tc.tile_pool(name="xn16", bufs=3))
            sco = ctx.enter_context(tc.tile_pool(name="sco", bufs=2))
            ps_r = ctx.enter_context(tc.tile_pool(name="ps_r", bufs=2,
                                                  space="PSUM"))
            ps_z = ctx.enter_context(tc.tile_pool(name="ps_z", bufs=2,
                                                  space="PSUM"))
            ps_x = ctx.enter_context(tc.tile_pool(name="ps_x", bufs=1,
                                                  space="PSUM"))
            ps_h = ctx.enter_context(tc.tile_pool(name="ps_h", bufs=1,
                                                  space="PSUM"))
            ps_f = ctx.enter_context(tc.tile_pool(name="ps_f", bufs=2,
                                                  space="PSUM"))

            # ---- tiny consts first: identity (warmup + epilogue), fc2 ----
            idt = cpool.tile([128, 128], FP32, tag="idt")
            nc.sync.dma_start(idt[:], dl["ident"].ap())
            f2t = cpool2.tile([SH, 1], FP16, tag="f2t")
            nc.sync.dma_start(f2t[:], dl["f2t16"].ap())
            # touch every activation function once so the ACT table loads
            # happen during the const-DMA wait, not on the scan chain
            warm = cpool2.tile([1, 4], FP32, tag="warm")
            for af in (AF.Sigmoid, AF.Tanh, AF.Copy):
                nc.scalar.activation(warm[:], idt[0:1, 0:4], af)
            # PE p-state warmup: fp32 identity matmuls (512 cyc each), no
            # data deps beyond idt, so they run while the big DMAs stream.
            # Aliases the fc1 psum tag: PE is in-order, so no extra blocking.
            wps = ps_f.tile([128, SGRP * BPC], FP32, tag="pps", name="wps")
            for i in range(N_WARMUP):
                nc.tensor.matmul(wps[:, 0:128], idt[:], idt[:],
                                 start=(i == 0), stop=(i == N_WARMUP - 1))

            # ---- fc1 weights, then feat g0 (quartered) so scoring starts
            # as early as possible ----
            f1h = cpool2.tile([128, FK // 2, 2, 128], FP8, tag="f1h")
            nc.sync.dma_start(f1h[:], dl["f18h"].ap())
            f1l = cpool2.tile([128, FK // 2, 2, 128], FP8, tag="f1l")
            nc.sync.dma_start(f1l[:], dl["f18l"].ap())

            # ================= helpers =================
            def load_c(s):
                ch = cstr.tile([128, 2, 2, BPC], FP8, tag="csh")
                nc.sync.dma_start(ch[:], c_t.ap()[0, s])
                cl = cstr.tile([128, 2, 2, BPC], FP8, tag="csl")
                nc.sync.dma_start(cl[:], c_t.ap()[1, s])
                return (ch, cl)

            def x_mms(cts, banks, with_stop):
                """x-part matmuls for one sentence into the given
                [(psum, gate)] banks. pr/pz use per-jj-half accumulation
                regions (the chain's sigmoid chunks read each half as soon
                as the h-part closes it); pxn is a single region, closed
                here (the h-part never accumulates into it)."""
                ch, cl = cts
                for pb, gate in banks:
                    for jj in range(4):
                        gt_ = gate * 4 + jj
                        for pair in range(2):
                            terms = [(wih_h, ch), (wih_h, cl),
                                     (wih_l, ch)]
                            for ti, (wt_, ct_) in enumerate(terms):
                                nc.tensor.matmul(
                                    pb[:, jj],
                                    wt_[:, pair, gt_],
                                    ct_[:, pair],
                                    start=(jj == 0 and pair == 0
                                           and ti == 0),
                                    stop=((with_stop or gate == 2)
                                          and jj == 3
                                          and pair == 1 and ti == 2),
                                    perf_mode=PM.DoubleRow)

            def h_mms(h8h_t, pr, pz, pxn, phn):
                """h-part fp8 DR matmuls: every gate is 2-term weight-side
                compensated ((Whi+Wlo) @ hhi), so only the hi plane of h is
                needed — the chain never produces a lo plane and the n-gate
                psum closes right after the hi plane lands. Gate order
                r, n, z: r feeds the sigmoid, n closes phn for tn, z last."""
                for gate, pb in ((0, pr), (2, phn), (1, pz)):
                    # r gate: 1 term (Whi @ hhi) — halving its matmul block
                    # lets sigmoid(r), which is on the serial loop, start
                    # ~200ns earlier. z/n: 2-term weight-side.
                    terms = [whh_h] if gate == 0 else [whh_h, whh_l]
                    for ti, wt_ in enumerate(terms):
                        for pair in (1, 0):
                            for jj in range(4):
                                nc.tensor.matmul(
                                    pb[:, jj], wt_[:, pair, 4 * gate + jj],
                                    h8h_t[:, 2 * pair:2 * pair + 2],
                                    start=(gate == 2 and ti == 0
                                           and pair == 1 and jj == 0),
                                    stop=(ti == len(terms) - 1
                                          and pair == 0 and jj == 3),
                                    perf_mode=PM.DoubleRow)

            # ---- scoring machinery ----
            grp = {}

            def load_feat(gi, quarters=False):
                # split loads along k so downstream fc1 chunks unblock
                # progressively; interleave hi/lo quarters for the startup
                # groups so chunk q unblocks after 1/4 of the bytes
                fh = fstr.tile([128, FK // 2, 2, SGRP * BPC], FP8,
                               tag="feath")
                fl = fstr.tile([128, FK // 2, 2, SGRP * BPC], FP8,
                               tag="featl")
                if quarters:
                    for q0 in range(0, 8, 2):
                        nc.sync.dma_start(fh[:, q0:q0 + 2],
                                          f_t.ap()[0, gi, :, q0:q0 + 2])
                        nc.sync.dma_start(fl[:, q0:q0 + 2],
                                          f_t.ap()[1, gi, :, q0:q0 + 2])
                else:
                    nc.sync.dma_start(fh[:, 0:4], f_t.ap()[0, gi, :, 0:4])
                    nc.sync.dma_start(fh[:, 4:8], f_t.ap()[0, gi, :, 4:8])
                    nc.sync.dma_start(fl[:, 0:4], f_t.ap()[1, gi, :, 0:4])
                    nc.sync.dma_start(fl[:, 4:8], f_t.ap()[1, gi, :, 4:8])
                grp[gi] = {"feat": (fh, fl)}

            def fc1_chunk(gi, q):
                """Quarter q of group gi's fc1 matmuls."""
                st = grp[gi]
                if q == 0:
                    st["pps"] = ps_f.tile([128, SGRP * BPC], FP32, tag="pps",
                                          name="pps")
                pps = st["pps"]
                fh, fl = st["feat"]
                for pair in range(2 * q, 2 * q + 2):
                    terms = [(f1h, fh), (f1h, fl), (f1l, fh)]
                    for ti, (wt_, ft_) in enumerate(terms):
                        nc.tensor.matmul(
                            pps[:], wt_[:, pair], ft_[:, pair],
                            start=(pair == 0 and ti == 0),
                            stop=(pair == FK // 2 - 1 and ti == 2),
                            perf_mode=PM.DoubleRow)
                if q == 3:
                    finish_group(gi)

            def finish_group(gi):
                st = grp[gi]
                pps = st["pps"]
                h1 = sco.tile([SH, SGRP * BPC], FP16, tag="h1")
                nc.scalar.activation(h1[:], pps[0:SH, :], AF.Tanh,
                                     scale=1.0 / 128)
                nc.tensor.matmul(pps[0:1, :], f2t[:], h1[:],
                                 start=True, stop=True)
                gt = gpool.tile([1, SGRP * BPC], FP16, tag="gt")
                nc.scalar.activation(gt[:], pps[0:1, :], AF.Sigmoid)
                gd = gdram.tile([1, SGRP * BPC], FP16, tag="gd")
                nc.sync.dma_start(gd[:], gt[:])
                grep = gpool.tile([128, SGRP, BPC], FP16, tag="grep")
                nc.sync.dma_start(
                    grep[:], gd[:].broadcast_to([128, SGRP * BPC]))
                st["grep"] = grep
                del st["pps"], st["feat"]

            # ================= prologue =================
            load_feat(0, quarters=True)
            for q in range(4):
                fc1_chunk(0, q)

            wih_h = cpool.tile([128, 2, NT, 2, 128], FP8, tag="wih_h")
            nc.sync.dma_start(wih_h[:], dl["wih8h"].ap())
            wih_l = cpool.tile([128, 2, NT, 2, 128], FP8, tag="wih_l")
            nc.sync.dma_start(wih_l[:], dl["wih8l"].ap())
            cbuf = {0: load_c(0)}
            pz = ps_z.tile([128, 4, BPC], FP32, tag="pz", name="pz")
            pxn = ps_x.tile([128, 4, BPC], FP32, tag="pxn", name="pxn")
            x_mms(cbuf[0], [(pxn, 2), (pz, 1)], with_stop=True)
            del cbuf[0]

            # recurrent weights right behind c0 (step 1's h-part gates on
            # whh_h); c1/c2 slot between the two planes so early x-matmuls
            # are not starved; group 1's scoring runs inside scan steps 0-3
            whh_h = cpool.tile([128, 2, NT, 2, 128], FP8, tag="whh_h")
            nc.sync.dma_start(whh_h[:], dl["whh8h"].ap())
            cbuf[1] = load_c(1)
            whh_l = cpool.tile([128, 2, NT, 2, 128], FP8, tag="whh_l")
            nc.sync.dma_start(whh_l[:], dl["whh8l"].ap())
            for s in range(2, 5):
                cbuf[s] = load_c(s)
            load_feat(1, quarters=True)
            load_feat(2)

            h16 = None
            h8h_t = None
            pr = None
            ISC = 1.0 / 512

            # ================= scan =================
            for s in range(S):
                # ---- h-part matmuls (s>0) ----
                if s > 0:
                    phn = ps_h.tile([128, 4, BPC], FP32, tag="phn",
                                    name="phn")
                    h_mms(h8h_t, pr, pz, pxn, phn)

                # ---- elementwise chain ----
                gi = s // SGRP
                j = s % SGRP
                grep = grp[gi]["grep"]

                if s > 0:
                    r_sb = ew.tile([128, 4, BPC], FP16, tag="r")
                    nc.scalar.activation(r_sb[:], pr[:], AF.Sigmoid,
                                         scale=ISC)
                    # tn/tn2 in fp16 at the 512x psum scale: tn2's second
                    # operand is the pre-copied fp16 xn, so the add runs in
                    # DVE fast mode instead of reading psum at 1x
                    tn = ew2.tile([128, 4, BPC], FP16, tag="tn")
                    nc.vector.tensor_tensor(tn[:], r_sb[:], phn[:], OP.mult)
                w_sb = ew.tile([128, 4, BPC], FP16, tag="w")
                nc.scalar.activation(w_sb[:], pz[:], AF.Sigmoid, scale=-ISC)
                if s > 0:
                    tn2 = ew2.tile([128, 4, BPC], FP16, tag="tn2")
                    nc.vector.tensor_tensor(tn2[:], tn[:], x16n[:], OP.add)
                a_sb = ew.tile([128, 4, BPC], FP16, tag="a")
                nc.vector.tensor_tensor(
                    a_sb[:], w_sb[:],
                    grep[:, j].unsqueeze(1).broadcast_to([128, 4, BPC]),
                    OP.mult)
                # a / t1 / p are off the critical chain (they fill the DVE
                # while tanh runs on ACT)
                if s > 0:
                    t1 = ew2.tile([128, 4, BPC], FP16, tag="t1")
                    nc.vector.tensor_tensor(t1[:], a_sb[:], h16[:], OP.mult)
                    p_sb = ew2.tile([128, 4, BPC], FP16, tag="p")
                    nc.vector.tensor_tensor(p_sb[:], h16[:], t1[:],
                                            OP.subtract)
                n_sb = ew.tile([128, 4, BPC], FP16, tag="n")
                if s > 0:
                    nc.scalar.activation(n_sb[:], tn2[:], AF.Tanh,
                                         scale=ISC)
                else:
                    nc.scalar.activation(n_sb[:], pxn[:], AF.Tanh,
                                         scale=ISC)
                last = s == S - 1
                if last:
                    nh = ew.tile([128, KH, BPC], FP32, tag="hf")
                    nhh = None
                else:
                    nh = hpool.tile([128, KH, BPC], FP16, tag="h", name="h")
                    nhh = hq.tile([128, KH, BPC], FP8, tag="h8h",
                                  name="h8h")
                # post-tanh tail, all on DVE (no cross-engine hops): the fp8
                # hi-plane adds come FIRST — they alone unblock ALL of next
                # step's h-part matmuls; the fp16 adds trail
                mh = {}
                for half in (1, 0):
                    k0 = 2 * half
                    if s > 0:
                        m_h = ew2.tile([128, 2, BPC], FP16, tag=f"m{half}",
                                       name=f"m{half}")
                        nc.vector.tensor_tensor(
                            m_h[:], a_sb[:, k0:k0 + 2], n_sb[:, k0:k0 + 2],
                            OP.mult)
                        mh[half] = m_h
                        if not last:
                            nc.vector.tensor_tensor(
                                nhh[:, k0:k0 + 2], p_sb[:, k0:k0 + 2],
                                m_h[:], OP.add)
                    elif not last:
                        nc.vector.tensor_tensor(
                            nhh[:, k0:k0 + 2], a_sb[:, k0:k0 + 2],
                            n_sb[:, k0:k0 + 2], OP.mult)
                for half in (1, 0):
                    k0 = 2 * half
                    if s == 0:
                        nc.vector.tensor_tensor(
                            nh[:, k0:k0 + 2], a_sb[:, k0:k0 + 2],
                            n_sb[:, k0:k0 + 2], OP.mult)
                    else:
                        nc.vector.tensor_tensor(
                            nh[:, k0:k0 + 2], p_sb[:, k0:k0 + 2],
                            mh[half][:], OP.add)
                h16 = nh
                h8h_t = nhh

                # ---- x-part matmuls for s+1 ----
                if s + 1 < S:
                    pr = ps_r.tile([128, 4, BPC], FP32, tag="pr", name="pr")
                    pz = ps_z.tile([128, 4, BPC], FP32, tag="pz", name="pz")
                    pxn = ps_x.tile([128, 4, BPC], FP32, tag="pxn",
                                    name="pxn")
                    x_mms(cbuf[s + 1], [(pr, 0), (pxn, 2), (pz, 1)],
                          with_stop=False)
                    # stage the n-gate x psum to fp16 SBUF off the critical
                    # path (tn2 then runs in DVE fast mode)
                    x16n = xnp.tile([128, 4, BPC], FP16, tag="x16n",
                                    name="x16n")
                    nc.scalar.activation(x16n[:], pxn[:], AF.Copy)
                    del cbuf[s + 1]
                    if s + 5 < S:
                        cbuf[s + 5] = load_c(s + 5)

                # ---- interleaved scoring ----
                # group 1 (needed from s=4) is scored during steps 0-3
                if s < 4:
                    fc1_chunk(1, s)
                gisc = s // SGRP + 2
                q = s % SGRP
                if gisc <= NGRP - 1:
                    if q == 0 and gisc + 1 <= NGRP - 1:
                        load_feat(gisc + 1)
                    fc1_chunk(gisc, q)

            # ================= epilogue =================
            # transpose + copy + store per kt-half as the last chain's
            # halves land
            pt = ps_r.tile([128, 4, BPC], FP32, tag="pr", name="ptr")
            ot = ew.tile([128, H], FP32, tag="ot")
            for half in (1, 0):
                for kt in (2 * half, 2 * half + 1):
                    nc.tensor.transpose(pt[:, kt], h16[:, kt], idt[:])
                nc.scalar.activation(
                    ot[:, 256 * half:256 * half + 256],
                    pt[:, 2 * half:2 * half + 2].rearrange(
                        "p k b -> p (k b)"), AF.Copy)
                nc.sync.dma_start(
                    out.ap()[:, 256 * half:256 * half + 256],
                    ot[:, 256 * half:256 * half + 256])

    nc.compile()
    return nc


def _prep(C, Q, prev_M, fc1_w, fc2_w, W_ih, W_hh):
    """Host-side layout/dtype transforms + per-core sharding."""
    consts = {}
    consts["ident"] = np.eye(128, dtype=np.float32)
    # W_ih^T hi/lo fp8 in DR lhsT layout [p, pair, gate-tile, i, 128]
    hi, lo = _split8(np.ascontiguousarray(W_ih.T), 16.0)  # [K, G3]
    for nm, t in (("wih8h", hi), ("wih8l", lo)):
        consts[nm] = np.ascontiguousarray(
            t.reshape(2, 2, 128, NT, 128).transpose(2, 0, 3, 1, 4))
    # W_hh^T * 512 hi/lo fp8, same layout (h quantized at scale 1.0)
    hi, lo = _split8(np.ascontiguousarray(W_hh.T), 512.0)  # [H, G3]
    for nm, t in (("whh8h", hi), ("whh8l", lo)):
        consts[nm] = np.ascontiguousarray(
            t.reshape(2, 2, 128, NT, 128).transpose(2, 0, 3, 1, 4))
    f1p = np.zeros((128, 4 * H), np.float32)  # pad SH 120 -> 128
    f1p[:SH] = fc1_w
    hi, lo = _split8(np.ascontiguousarray(f1p.T), 16.0)  # [K, 128]
    for nm, t in (("f18h", hi), ("f18l", lo)):
        consts[nm] = np.ascontiguousarray(
            t.reshape(FK // 2, 2, 128, 128).transpose(2, 0, 1, 3))
    consts["f2t16"] = np.ascontiguousarray(fc2_w.T).astype(F16)

    in_maps = []
    for c in range(NCORES):
        lo_, hi_ = c * BPC, (c + 1) * BPC
        Cc = np.ascontiguousarray(C[lo_:hi_])          # [BPC, S, H]
        Qc = Q[lo_:hi_, 0]                              # [BPC, H]
        Mc = prev_M[lo_:hi_, 0]
        m = {}
        Ct = np.ascontiguousarray(Cc.transpose(1, 2, 0))   # [S, H, BPC]
        chi, clo = _split8(Ct, 32.0)  # [S, H, BPC]
        both = np.stack([chi, clo])   # [2, S, H, BPC]
        m["c8"] = np.ascontiguousarray(
            both.reshape(2, S, 2, 2, 128, BPC).transpose(0, 1, 4, 2, 3, 5))
        # feat [BPC, S, 4H]
        feat = np.concatenate(
            [Cc * Qc[:, None, :], Cc * Mc[:, None, :],
             np.abs(Cc - Qc[:, None, :]), np.abs(Cc - Mc[:, None, :])],
            axis=2)
        # -> [NGRP, 4H, SGRP, BPC]
        ftr = np.ascontiguousarray(
            feat.transpose(1, 2, 0).reshape(NGRP, SGRP, 4 * H, BPC)
            .transpose(0, 2, 1, 3))
        fhi, flo = _split8(ftr, 8.0)       # [NGRP, 4H, SGRP, BPC]
        both = np.stack([fhi, flo])
        m["feat8"] = np.ascontiguousarray(
            both.reshape(2, NGRP, FK // 2, 2, 128, SGRP * BPC)
            .transpose(0, 1, 4, 2, 3, 5))
        in_maps.append(m)
    return consts, in_maps


def kernel(C, Q, prev_M, fc1_w, fc1_b, fc2_w, fc2_b, W_ih, W_hh, b_ih, b_hh):
    from concourse.bass_utils import run_bass_kernel_spmd

    C = np.asarray(C, dtype=np.float32)
    Q = np.asarray(Q, dtype=np.float32)
    prev_M = np.asarray(prev_M, dtype=np.float32)
    fc1_w = np.asarray(fc1_w, np.float32)
    fc2_w = np.asarray(fc2_w, np.float32)
    W_ih = np.asarray(W_ih, np.float32)
    W_hh = np.asarray(W_hh, np.float32)
    fc1_b = np.asarray(fc1_b, np.float32)
    fc2_b = np.asarray(fc2_b, np.float32)
    b_ih = np.asarray(b_ih, np.float32)
    b_hh = np.asarray(b_hh, np.float32)
    assert not (np.any(fc1_b) or np.any(fc2_b) or np.any(b_ih)
                or np.any(b_hh)), "nonzero biases unsupported in v3"

    consts, in_maps = _prep(C, Q, prev_M, fc1_w, fc2_w, W_ih, W_hh)

    key = tuple(np.asarray(v).tobytes() for v in consts.values())
    kh = hash(key)
    if kh not in _CACHE:
        _CACHE[kh] = _build(consts)
    nc = _CACHE[kh]

    res = run_bass_kernel_spmd(nc, in_maps, list(range(NCORES)))
    h = np.concatenate([res.results[c]["out"] for c in range(NCORES)],
                       axis=0)
    return h[:, None, :].astype(np.float32)


# revision 46
# speedup vs baseline: 1.2224x; 1.0128x over previous
"""EpisodicMemory Trainium2 kernel, v3.

Data-parallel over batch across 8 NeuronCores (128 batch rows per core).

Layout is "flipped": the GRU state h and all gate pre-activations live as
[H-on-partitions (4 k-tiles of 128), batch-on-free(128)] tiles, so the
recurrent matmul h @ W_hh^T needs NO transposes: its rhs (moving operand)
is h itself, and the elementwise update produces h directly in that
layout. Per-sentence episodic gates are broadcast across partitions with a
stride-0 DMA from DRAM.

Precision plan (validated offline, rel err ~1.3e-2 vs 2e-2 budget):
 - x-part (C @ W_ih^T) and scoring fc1: 3-term error-compensated fp8-e4m3
   DoubleRow matmuls (hi/lo splits of both operands, Whi@Chi + Whi@Clo +
   Wlo@Chi), 0.5 cyc/row with K=256 per instruction.
 - h-part (h @ W_hh^T): fp8 DoubleRow, every gate 2-term WEIGHT-side
   compensated ((Whi+Wlo) @ hhi) so only the hi fp8 plane of h is ever
   needed. h stays in fp16 (quantized at scale 1.0, |h|<=1); the chain's
   p+m add is issued twice, fp8-out first (h8h) so the next step's h-part
   matmuls unblock before the fp16 adds complete.
 - tn/tn2 (r*hn + xn) are fp16 at the 512x psum scale; the n-gate x psum
   is pre-staged to fp16 SBUF (ACT copy, off the critical path) so tn2
   runs in the DVE 2x mode instead of reading psum at 1x.
 - All W matrices pre-scaled so every GRU psum holds 512x the preact
   (fc1: 128x); the 1/512 folds into the ACT sigmoid/tanh scale.
 - C / feat = [C*Q, C*prev_M, |C-Q|, |C-prev_M|] quantized host-side.

Per-step critical loop (the scan is latency-bound, not engine-bound):
h8h -> r/n/z h-matmuls -> sigmoid(r) -> tn -> tn2 -> tanh (2 chunks) ->
m -> h8h. DVE work is kept just under the loop period; a/t1/p fill the
tanh window. Startup: tiny consts DMA first, then fc1 weights + feat
group 0 in interleaved hi/lo quarters so scoring matmuls start ~4us in;
identity-tile warmup matmuls ramp the PE p-state during the DMA wait.
"""
import numpy as np
import ml_dtypes

H = 512
SH = 120
B = 1024
S = 64
NCORES = 8
BPC = B // NCORES  # 128
KH = H // 128      # 4
G3 = 3 * H
SGRP = 4
NGRP = S // SGRP   # 16
NT = G3 // 128     # 12 gate tiles
FK = 4 * H // 128  # 16 feat k-tiles
E4 = ml_dtypes.float8_e4m3
F16 = np.float16

N_WARMUP = 10      # identity matmuls to ramp the PE p-state during DMA wait

_CACHE = {}


def _q8(x):
    return np.clip(np.asarray(x, np.float32), -240.0, 240.0).astype(E4)


def _split8(x, scale):
    hi = _q8(x * scale)
    lo = _q8(x * scale - hi.astype(np.float32))
    return hi, lo


def _build(consts):
    import concourse.bass as bass
    import concourse.tile as tile
    from concourse import bacc, mybir

    FP32 = mybir.dt.float32
    FP16 = mybir.dt.float16
    FP8 = mybir.dt.float8e4
    OP = mybir.AluOpType
    AF = mybir.ActivationFunctionType
    PM = mybir.MatmulPerfMode

    nc = bacc.Bacc("TRN2", target_bir_lowering=False, debug=False,
                   num_devices=NCORES)

    # ---- external inputs (per core) ----
    c_t = nc.dram_tensor("c8", [2, S, 128, 2, 2, BPC], FP8,
                         kind="ExternalInput")  # [hi/lo, s, p, pair, i, b]
    f_t = nc.dram_tensor("feat8", [2, NGRP, 128, FK // 2, 2, SGRP * BPC],
                         FP8, kind="ExternalInput")
    out = nc.dram_tensor("out", [BPC, H], FP32, kind="ExternalOutput")

    # ---- inline consts ----
    dl = {}
    for k, v in consts.items():
        dl[k] = nc.inline_tensor(v, name=k)

    from contextlib import ExitStack
    with tile.TileContext(nc) as tc:
        with ExitStack() as ctx:
            cpool = ctx.enter_context(tc.tile_pool(name="const", bufs=1))
            cpool2 = ctx.enter_context(tc.tile_pool(name="const2", bufs=1))
            cstr = ctx.enter_context(tc.tile_pool(name="cstr", bufs=6))
            fstr = ctx.enter_context(tc.tile_pool(name="fstr", bufs=3))
            hpool = ctx.enter_context(tc.tile_pool(name="h", bufs=4))
            hq = ctx.enter_context(tc.tile_pool(name="hq", bufs=4))
            gpool = ctx.enter_context(tc.tile_pool(name="g", bufs=4))
            gdram = ctx.enter_context(tc.tile_pool(name="gd", bufs=4,
                                                   space="DRAM"))
            ew = ctx.enter_context(tc.tile_pool(name="ew", bufs=5))
            ew2 = ctx.enter_context(tc.tile_pool(name="ew2", bufs=5))
            xnp = ctx.enter_context(tc.tile_pool(name="xn16", bufs=4))
            sco = ctx.enter_context(tc.tile_pool(name="sco", bufs=2))
            ps_r = ctx.enter_context(tc.tile_pool(name="ps_r", bufs=2,
                                                  space="PSUM"))
            ps_z = ctx.enter_context(tc.tile_pool(name="ps_z", bufs=2,
                                                  space="PSUM"))
            ps_x = ctx.enter_context(tc.tile_pool(name="ps_x", bufs=1,
                                                  space="PSUM"))
            ps_h = ctx.enter_context(tc.tile_pool(name="ps_h", bufs=1,
                                                  space="PSUM"))
            ps_f = ctx.enter_context(tc.tile_pool(name="ps_f", bufs=2,
                                                  space="PSUM"))

            # ---- tiny consts first: identity (warmup + epilogue), fc2 ----
            idt = cpool.tile([128, 128], FP32, tag="idt")
            nc.sync.dma_start(idt[:], dl["ident"].ap())
            f2t = cpool2.tile([SH, 1], FP16, tag="f2t")
            nc.sync.dma_start(f2t[:], dl["f2t16"].ap())
            # touch every activation function once so the ACT table loads
            # happen during the const-DMA wait, not on the scan chain
            warm = cpool2.tile([1, 4], FP32, tag="warm")
            for af in (AF.Sigmoid, AF.Tanh, AF.Copy):
                nc.scalar.activation(warm[:], idt[0:1, 0:4], af)
            # PE p-state warmup: fp32 identity matmuls (512 cyc each), no
            # data deps beyond idt, so they run while the big DMAs stream.
            # Aliases the fc1 psum tag: PE is in-order, so no extra blocking.
            wps = ps_f.tile([128, SGRP * BPC], FP32, tag="pps", name="wps")
            for i in range(N_WARMUP):
                nc.tensor.matmul(wps[:, 0:128], idt[:], idt[:],
                                 start=(i == 0), stop=(i == N_WARMUP - 1))

            # ---- fc1 weights, then feat g0 (quartered) so scoring starts
            # as early as possible ----
            f1h = cpool2.tile([128, FK // 2, 2, 128], FP8, tag="f1h")
            nc.sync.dma_start(f1h[:], dl["f18h"].ap())
            f1l = cpool2.tile([128, FK // 2, 2, 128], FP8, tag="f1l")
            nc.sync.dma_start(f1l[:], dl["f18l"].ap())

            # ================= helpers =================
            def load_c(s):
                ch = cstr.tile([128, 2, 2, BPC], FP8, tag="csh")
                nc.sync.dma_start(ch[:], c_t.ap()[0, s])
                cl = cstr.tile([128, 2, 2, BPC], FP8, tag="csl")
                nc.sync.dma_start(cl[:], c_t.ap()[1, s])
                return (ch, cl)

            def x_mms(cts, banks, with_stop):
                """x-part matmuls for one sentence into the given
                [(psum, gate)] banks. pr/pz use per-jj-half accumulation
                regions (the chain's sigmoid chunks read each half as soon
                as the h-part closes it); pxn is a single region, closed
                here (the h-part never accumulates into it)."""
                ch, cl = cts
                for pb, gate in banks:
                    for jj in range(4):
                        gt_ = gate * 4 + jj
                        for pair in range(2):
                            terms = [(wih_h, ch), (wih_h, cl),
                                     (wih_l, ch)]
                            for ti, (wt_, ct_) in enumerate(terms):
                                nc.tensor.matmul(
                                    pb[:, jj],
                                    wt_[:, pair, gt_],
                                    ct_[:, pair],
                                    start=(jj == 0 and pair == 0
                                           and ti == 0),
                                    stop=((with_stop or gate == 2)
                                          and jj == 3
                                          and pair == 1 and ti == 2),
                                    perf_mode=PM.DoubleRow)

            def h_mms(h8h_t, pr, pz, pxn, phn):
                """h-part fp8 DR matmuls: every gate is 2-term weight-side
                compensated ((Whi+Wlo) @ hhi), so only the hi plane of h is
                needed — the chain never produces a lo plane and the n-gate
                psum closes right after the hi plane lands. Gate order
                r, n, z: r feeds the sigmoid, n closes phn for tn, z last."""
                for gate, pb in ((0, pr), (2, phn), (1, pz)):
                    # r gate: 1 term (Whi @ hhi) — halving its matmul block
                    # lets sigmoid(r), which is on the serial loop, start
                    # ~200ns earlier. z/n: 2-term weight-side.
                    terms = [whh_h] if gate == 0 else [whh_h, whh_l]
                    for ti, wt_ in enumerate(terms):
                        for pair in (1, 0):
                            for jj in range(4):
                                nc.tensor.matmul(
                                    pb[:, jj], wt_[:, pair, 4 * gate + jj],
                                    h8h_t[:, 2 * pair:2 * pair + 2],
                                    start=(gate == 2 and ti == 0
                                           and pair == 1 and jj == 0),
                                    stop=(ti == len(terms) - 1
                                          and pair == 0 and jj == 3),
                                    perf_mode=PM.DoubleRow)

            # ---- scoring machinery ----
            grp = {}

            def load_feat(gi, quarters=False):
                # split loads along k so downstream fc1 chunks unblock
                # progressively; interleave hi/lo quarters for the startup
                # groups so chunk q unblocks after 1/4 of the bytes
                fh = fstr.tile([128, FK // 2, 2, SGRP * BPC], FP8,
                               tag="feath")
                fl = fstr.tile([128, FK // 2, 2, SGRP * BPC], FP8,
                               tag="featl")
                if quarters:
                    for q0 in range(0, 8, 2):
                        nc.sync.dma_start(fh[:, q0:q0 + 2],
                                          f_t.ap()[0, gi, :, q0:q0 + 2])
                        nc.sync.dma_start(fl[:, q0:q0 + 2],
                                          f_t.ap()[1, gi, :, q0:q0 + 2])
                else:
                    nc.sync.dma_start(fh[:, 0:4], f_t.ap()[0, gi, :, 0:4])
                    nc.sync.dma_start(fh[:, 4:8], f_t.ap()[0, gi, :, 4:8])
                    nc.sync.dma_start(fl[:, 0:4], f_t.ap()[1, gi, :, 0:4])
                    nc.sync.dma_start(fl[:, 4:8], f_t.ap()[1, gi, :, 4:8])
                grp[gi] = {"feat": (fh, fl)}

            def fc1_chunk(gi, q):
                """Quarter q of group gi's fc1 matmuls."""
                st = grp[gi]
                if q == 0:
                    st["pps"] = ps_f.tile([128, SGRP * BPC], FP32, tag="pps",
                                          name="pps")
                pps = st["pps"]
                fh, fl = st["feat"]
                for pair in range(2 * q, 2 * q + 2):
                    terms = [(f1h, fh), (f1h, fl), (f1l, fh)]
                    for ti, (wt_, ft_) in enumerate(terms):
                        nc.tensor.matmul(
                            pps[:], wt_[:, pair], ft_[:, pair],
                            start=(pair == 0 and ti == 0),
                            stop=(pair == FK // 2 - 1 and ti == 2),
                            perf_mode=PM.DoubleRow)
                if q == 3:
                    finish_group(gi)

            def finish_group(gi):
                st = grp[gi]
                pps = st["pps"]
                h1 = sco.tile([SH, SGRP * BPC], FP16, tag="h1")
                nc.scalar.activation(h1[:], pps[0:SH, :], AF.Tanh,
                                     scale=1.0 / 128)
                nc.tensor.matmul(pps[0:1, :], f2t[:], h1[:],
                                 start=True, stop=True)
                gt = gpool.tile([1, SGRP * BPC], FP16, tag="gt")
                nc.scalar.activation(gt[:], pps[0:1, :], AF.Sigmoid)
                gd = gdram.tile([1, SGRP * BPC], FP16, tag="gd")
                nc.sync.dma_start(gd[:], gt[:])
                grep = gpool.tile([128, SGRP, BPC], FP16, tag="grep")
                nc.sync.dma_start(
                    grep[:], gd[:].broadcast_to([128, SGRP * BPC]))
                st["grep"] = grep
                del st["pps"], st["feat"]

            # ================= prologue =================
            load_feat(0, quarters=True)
            for q in range(4):
                fc1_chunk(0, q)

            wih_h = cpool.tile([128, 2, NT, 2, 128], FP8, tag="wih_h")
            nc.sync.dma_start(wih_h[:], dl["wih8h"].ap())
            wih_l = cpool.tile([128, 2, NT, 2, 128], FP8, tag="wih_l")
            nc.sync.dma_start(wih_l[:], dl["wih8l"].ap())
            cbuf = {0: load_c(0)}
            pz = ps_z.tile([128, 4, BPC], FP32, tag="pz", name="pz")
            pxn = ps_x.tile([128, 4, BPC], FP32, tag="pxn", name="pxn")
            x_mms(cbuf[0], [(pxn, 2), (pz, 1)], with_stop=True)
            del cbuf[0]

            # recurrent weights right behind c0 (step 1's h-part gates on
            # whh_h); c1/c2 slot between the two planes so early x-matmuls
            # are not starved; group 1's scoring runs inside scan steps 0-3
            whh_h = cpool.tile([128, 2, NT, 2, 128], FP8, tag="whh_h")
            nc.sync.dma_start(whh_h[:], dl["whh8h"].ap())
            cbuf[1] = load_c(1)
            whh_l = cpool.tile([128, 2, NT, 2, 128], FP8, tag="whh_l")
            nc.sync.dma_start(whh_l[:], dl["whh8l"].ap())
            for s in range(2, 5):
                cbuf[s] = load_c(s)
            load_feat(1, quarters=True)
            load_feat(2)

            h16 = None
            h8h_t = None
            pr = None
            ISC = 1.0 / 512

            # ================= scan =================
            for s in range(S):
                # ---- h-part matmuls (s>0) ----
                if s > 0:
                    phn = ps_h.tile([128, 4, BPC], FP32, tag="phn",
                                    name="phn")
                    h_mms(h8h_t, pr, pz, pxn, phn)

                # ---- elementwise chain ----
                gi = s // SGRP
                j = s % SGRP
                grep = grp[gi]["grep"]

                if s > 0:
                    r_sb = ew.tile([128, 4, BPC], FP16, tag="r")
                    nc.scalar.activation(r_sb[:], pr[:], AF.Sigmoid,
                                         scale=ISC)
                    # tn/tn2 in fp16 at the 512x psum scale: tn2's second
                    # operand is the pre-copied fp16 xn, so the add runs in
                    # DVE fast mode instead of reading psum at 1x
                    tn = ew2.tile([128, 4, BPC], FP16, tag="tn")
                    nc.vector.tensor_tensor(tn[:], r_sb[:], phn[:], OP.mult)
                w_sb = ew.tile([128, 4, BPC], FP16, tag="w")
                nc.scalar.activation(w_sb[:], pz[:], AF.Sigmoid, scale=-ISC)
                if s > 0:
                    tn2 = ew2.tile([128, 4, BPC], FP16, tag="tn2")
                    nc.vector.tensor_tensor(tn2[:], tn[:], x16n[:], OP.add)
                a_sb = ew.tile([128, 4, BPC], FP16, tag="a")
                nc.vector.tensor_tensor(
                    a_sb[:], w_sb[:],
                    grep[:, j].unsqueeze(1).broadcast_to([128, 4, BPC]),
                    OP.mult)
                # a / t1 / p are off the critical chain (they fill the DVE
                # while tanh runs on ACT)
                if s > 0:
                    t1 = ew2.tile([128, 4, BPC], FP16, tag="t1")
                    nc.vector.tensor_tensor(t1[:], a_sb[:], h16[:], OP.mult)
                    p_sb = ew2.tile([128, 4, BPC], FP16, tag="p")
                    nc.vector.tensor_tensor(p_sb[:], h16[:], t1[:],
                                            OP.subtract)
                n_sb = ew.tile([128, 4, BPC], FP16, tag="n")
                if s > 0:
                    nc.scalar.activation(n_sb[:], tn2[:], AF.Tanh,
                                         scale=ISC)
                else:
                    nc.scalar.activation(n_sb[:], pxn[:], AF.Tanh,
                                         scale=ISC)
                last = s == S - 1
                if last:
                    nh = ew.tile([128, KH, BPC], FP32, tag="hf")
                    nhh = None
                else:
                    nh = hpool.tile([128, KH, BPC], FP16, tag="h", name="h")
                    nhh = hq.tile([128, KH, BPC], FP8, tag="h8h",
                                  name="h8h")
                # post-tanh tail, all on DVE (no cross-engine hops): the fp8
                # hi-plane adds come FIRST — they alone unblock ALL of next
                # step's h-part matmuls; the fp16 adds trail
                mh = {}
                for half in (1, 0):
                    k0 = 2 * half
                    if s > 0:
                        m_h = ew2.tile([128, 2, BPC], FP16, tag=f"m{half}",
                                       name=f"m{half}")
                        nc.vector.tensor_tensor(
                            m_h[:], a_sb[:, k0:k0 + 2], n_sb[:, k0:k0 + 2],
                            OP.mult)
                        mh[half] = m_h
                        if not last:
                            nc.vector.tensor_tensor(
                                nhh[:, k0:k0 + 2], p_sb[:, k0:k0 + 2],
                                m_h[:], OP.add)
                    elif not last:
                        nc.vector.tensor_tensor(
                            nhh[:, k0:k0 + 2], a_sb[:, k0:k0 + 2],
                            n_sb[:, k0:k0 + 2], OP.mult)
                for half in (1, 0):
                    k0 = 2 * half
                    if s == 0:
                        nc.vector.tensor_tensor(
                            nh[:, k0:k0 + 2], a_sb[:, k0:k0 + 2],
                            n_sb[:, k0:k0 + 2], OP.mult)
                    else:
                        nc.vector.tensor_tensor(
                            nh[:, k0:k0 + 2], p_sb[:, k0:k0 + 2],
                            mh[half][:], OP.add)
                h16 = nh
                h8h_t = nhh

                # ---- x-part matmuls for s+1 ----
                if s + 1 < S:
                    pr = ps_r.tile([128, 4, BPC], FP32, tag="pr", name="pr")
                    pz = ps_z.tile([128, 4, BPC], FP32, tag="pz", name="pz")
                    pxn = ps_x.tile([128, 4, BPC], FP32, tag="pxn",
                                    name="pxn")
                    x_mms(cbuf[s + 1], [(pr, 0), (pxn, 2), (pz, 1)],
                          with_stop=False)
                    # stage the n-gate x psum to fp16 SBUF off the critical
                    # path (tn2 then runs in DVE fast mode)
                    x16n = xnp.tile([128, 4, BPC], FP16, tag="x16n",
                                    name="x16n")
                    nc.scalar.activation(x16n[:], pxn[:], AF.Copy)
                    del cbuf[s + 1]
                    if s + 5 < S:
                        cbuf[s + 5] = load_c(s + 5)

                # ---- interleaved scoring ----
                # group 1 (needed from s=4) is scored during steps 0-3
                if s < 4:
                    fc1_chunk(1, s)
                gisc = s // SGRP + 2
                q = s % SGRP
                if gisc <= NGRP - 1:
                    if q == 0 and gisc + 1 <= NGRP - 1:
                        load_feat(gisc + 1)
                    fc1_chunk(gisc, q)

            # ================= epilogue =================
            # transpose + copy + store per kt-half as the last chain's
            # halves land
            pt = ps_r.tile([128, 4, BPC], FP32, tag="pr", name="ptr")
            ot = ew.tile([128, H], FP32, tag="ot")
            for half in (1, 0):
                for kt in (2 * half, 2 * half + 1):
                    nc.tensor.transpose(pt[:, kt], h16[:, kt], idt[:])
                nc.scalar.activation(
                    ot[:, 256 * half:256 * half + 256],
                    pt[:, 2 * half:2 * half + 2].rearrange(
                        "p k b -> p (k b)"), AF.Copy)
                nc.sync.dma_start(
                    out.ap()[:, 256 * half:256 * half + 256],
                    ot[:, 256 * half:256 * half + 256])

    nc.compile()
    return nc


def _prep(C, Q, prev_M, fc1_w, fc2_w, W_ih, W_hh):
    """Host-side layout/dtype transforms + per-core sharding."""
    consts = {}
    consts["ident"] = np.eye(128, dtype=np.float32)
    # W_ih^T hi/lo fp8 in DR lhsT layout [p, pair, gate-tile, i, 128]
    hi, lo = _split8(np.ascontiguousarray(W_ih.T), 16.0)  # [K, G3]
    for nm, t in (("wih8h", hi), ("wih8l", lo)):
        consts[nm] = np.ascontiguousarray(
            t.reshape(2, 2, 128, NT, 128).transpose(2, 0, 3, 1, 4))
    # W_hh^T * 512 hi/lo fp8, same layout (h quantized at scale 1.0)
    hi, lo = _split8(np.ascontiguousarray(W_hh.T), 512.0)  # [H, G3]
    for nm, t in (("whh8h", hi), ("whh8l", lo)):
        consts[nm] = np.ascontiguousarray(
            t.reshape(2, 2, 128, NT, 128).transpose(2, 0, 3, 1, 4))
    f1p = np.zeros((128, 4 * H), np.float32)  # pad SH 120 -> 128
    f1p[:SH] = fc1_w
    hi, lo = _split8(np.ascontiguousarray(f1p.T), 16.0)  # [K, 128]
    for nm, t in (("f18h", hi), ("f18l", lo)):
        consts[nm] = np.ascontiguousarray(
            t.reshape(FK // 2, 2, 128, 128).transpose(2, 0, 1, 3))
    consts["f2t16"] = np.ascontiguousarray(fc2_w.T).astype(F16)

    in_maps = []
    for c in range(NCORES):
        lo_, hi_ = c * BPC, (c + 1) * BPC
        Cc = np.ascontiguousarray(C[lo_:hi_])          # [BPC, S, H]
        Qc = Q[lo_:hi_, 0]                              # [BPC, H]
        Mc = prev_M[lo_:hi_, 0]
        m = {}
        Ct = np.ascontiguousarray(Cc.transpose(1, 2, 0))   # [S, H, BPC]
        chi, clo = _split8(Ct, 32.0)  # [S, H, BPC]
        both = np.stack([chi, clo])   # [2, S, H, BPC]
        m["c8"] = np.ascontiguousarray(
            both.reshape(2, S, 2, 2, 128, BPC).transpose(0, 1, 4, 2, 3, 5))
        # feat [BPC, S, 4H]
        feat = np.concatenate(
            [Cc * Qc[:, None, :], Cc * Mc[:, None, :],
             np.abs(Cc - Qc[:, None, :]), np.abs(Cc - Mc[:, None, :])],
            axis=2)
        # -> [NGRP, 4H, SGRP, BPC]
        ftr = np.ascontiguousarray(
            feat.transpose(1, 2, 0).reshape(NGRP, SGRP, 4 * H, BPC)
            .transpose(0, 2, 1, 3))
        fhi, flo = _split8(ftr, 8.0)       # [NGRP, 4H, SGRP, BPC]
        both = np.stack([fhi, flo])
        m["feat8"] = np.ascontiguousarray(
            both.reshape(2, NGRP, FK // 2, 2, 128, SGRP * BPC)
            .transpose(0, 1, 4, 2, 3, 5))
        in_maps.append(m)
    return consts, in_maps


def kernel(C, Q, prev_M, fc1_w, fc1_b, fc2_w, fc2_b, W_ih, W_hh, b_ih, b_hh):
    from concourse.bass_utils import run_bass_kernel_spmd

    C = np.asarray(C, dtype=np.float32)
    Q = np.asarray(Q, dtype=np.float32)
    prev_M = np.asarray(prev_M, dtype=np.float32)
    fc1_w = np.asarray(fc1_w, np.float32)
    fc2_w = np.asarray(fc2_w, np.float32)
    W_ih = np.asarray(W_ih, np.float32)
    W_hh = np.asarray(W_hh, np.float32)
    fc1_b = np.asarray(fc1_b, np.float32)
    fc2_b = np.asarray(fc2_b, np.float32)
    b_ih = np.asarray(b_ih, np.float32)
    b_hh = np.asarray(b_hh, np.float32)
    assert not (np.any(fc1_b) or np.any(fc2_b) or np.any(b_ih)
                or np.any(b_hh)), "nonzero biases unsupported in v3"

    consts, in_maps = _prep(C, Q, prev_M, fc1_w, fc2_w, W_ih, W_hh)

    key = tuple(np.asarray(v).tobytes() for v in consts.values())
    kh = hash(key)
    if kh not in _CACHE:
        _CACHE[kh] = _build(consts)
    nc = _CACHE[kh]

    res = run_bass_kernel_spmd(nc, in_maps, list(range(NCORES)))
    h = np.concatenate([res.results[c]["out"] for c in range(NCORES)],
                       axis=0)
    return h[:, None, :].astype(np.float32)
